# revision 47
# baseline (speedup 1.0000x reference)
"""DGCNN (3x DynamicEdgeConv + global max pool + MLP head) on 8 Trainium2
NeuronCores, data-parallel over the batch (one point cloud per core).

EdgeConv algebra: h_ij = [x_i, x_j - x_i] @ W + b = u_i + v_j with
  u = x @ (Wa - Wb) + b,  v = x @ Wb;  out_i = u_i + max_{j in knn(i)} v_j.
kNN key: d_ij = x_i.x_j - |x_j|^2/2 (nearest = largest, self = row max).

Selection key: u32 = (fp16(d) << 16) | column_index.  The Act engine
converts the f32 PSUM distance row to fp16 directly into the high u16
halves of a persistent buffer whose low halves hold a static iota; the
u32 then compares as f32 exactly like the distance (ties break by index),
so no DVE bit-pack pass is needed.  DVE does top-8 of each 256-wide chunk
(8x max8) -> 64 candidates -> top-24 via 3x max8 + 2x match_replace;
slot 0 = self, slots 1..20 are the 20 nearest.  Chunked selection + fp16
keys are approximate; end-to-end rel err ~1.2e-2 on hw (gate 2e-2).

Gather: ONE dma_gather (SWDGE multi-index ucode, single_packet=False --
the single-packet path dies on hw above ~1k indices) per 128-point row
tile fetches all 20*128 neighbor rows from the fp16 v table in DRAM in
transpose mode, landing feature-major [128d, EC, (slot, point)].  Its
int16 index table wrapped[i%16, i//16] = flat[i], i = slot*128 + p,
replicated across the 8 Q7 core stripes, is produced by 8 one-hot "fold"
matmuls on PE (R[:, b, m] = jtab[16b + p%16, m]) plus an Act-engine
transposing f32->i16 convert.  Neighbor max is a 5-op fp16 tensor_tensor
tree on DVE (2x perf mode); out = u + vmax runs as two fp16 identity
matmuls accumulating into PSUM on the (idle) PE plus an Act copy, keeping
the adds off the DVE critical path.

The reduce of tile t runs DEPTH tiles behind its selection so the fold /
descgen / DMA / sem chain of the gather is fully hidden; the next layer's
v / u^T / nsq are produced inside the tile loop as h^T columns complete.
Global max pool is a free-axis tensor_reduce on [128, 2, N].

Engine budget at ~314us (TimelineSim): DVE ~260 (max8 144, tree ~95),
DMA engines 188 (gathers 175), Act ~175, PE ~150, Pool 93.
"""
import numpy as np

_NC_CACHE = {}
_DEPTHS = {1: 7, 2: 8, 3: 6}

N = 2048
NT = 16          # row tiles of 128 points
NCH = 8          # selection chunks per row (256 wide)
CHW = N // NCH
K = 20
NI = K * 128     # gather indices per row tile



def _builder(debug=False):
    import concourse.bacc as bacc
    import concourse.mybir as mybir
    from concourse.tile import TileContext

    F32 = mybir.dt.float32
    F32R = mybir.dt.float32r
    F16 = mybir.dt.float16
    I16 = mybir.dt.int16
    U32 = mybir.dt.uint32
    U16 = mybir.dt.uint16
    AF = mybir.ActivationFunctionType
    ALU = mybir.AluOpType
    AX = mybir.AxisListType

    def ts(i, s):
        return slice(i * s, (i + 1) * s)

    nc = bacc.Bacc("TRN2", num_devices=8)

    def stt_imm(eng, out, in0, imm, in1, op0, op1):
        """scalar_tensor_tensor with a uint32-typed immediate (the public
        helper hardcodes float32 imm, which the BIR verifier rejects for
        bitvec ops)."""
        return eng.add_instruction(
            mybir.InstTensorScalarPtr(
                name=eng.bass.get_next_instruction_name(),
                is_scalar_tensor_tensor=True,
                op0=op0,
                op1=op1,
                ins=[eng.lower_ap(in0),
                     mybir.ImmediateValue(dtype=mybir.dt.uint32, value=imm),
                     eng.lower_ap(in1)],
                outs=[eng.lower_ap(out)],
            ))

    def ts_imm(eng, out, in0, imm, op0):
        """tensor_scalar with a uint32-typed immediate."""
        return eng.add_instruction(
            mybir.InstTensorScalarPtr(
                name=eng.bass.get_next_instruction_name(),
                op0=op0,
                op1=mybir.AluOpType.bypass,
                ins=[eng.lower_ap(in0),
                     mybir.ImmediateValue(dtype=mybir.dt.uint32, value=imm)],
                outs=[eng.lower_ap(out)],
            ))

    xT = nc.dram_tensor("xT", [3, N], F32R, kind="ExternalInput").ap()
    nsq1_in = nc.dram_tensor("nsq1", [1, N], F32R, kind="ExternalInput").ap()
    v1_in = nc.dram_tensor("v1", [N, 128], F16, kind="ExternalInput").ap()
    uT1_in = nc.dram_tensor("uT1", [64, N], F16, kind="ExternalInput").ap()
    MB_in = nc.dram_tensor("MB", [128, 1024], F32R, kind="ExternalInput").ap()
    idnh_in = nc.dram_tensor("idnh", [128, 128], F16, kind="ExternalInput").ap()
    AB2 = nc.dram_tensor("AB2", [64, 128], F32R, kind="ExternalInput").ap()
    BB2 = nc.dram_tensor("BB2", [64, 128], F32R, kind="ExternalInput").ap()
    b2c = nc.dram_tensor("b2c", [128, 1], F32, kind="ExternalInput").ap()
    AB3 = nc.dram_tensor("AB3", [128, 256], F32R, kind="ExternalInput").ap()
    BB3 = nc.dram_tensor("BB3", [128, 256], F32R, kind="ExternalInput").ap()
    b3c = nc.dram_tensor("b3c", [128, 2], F32, kind="ExternalInput").ap()
    fc1w = nc.dram_tensor("fc1w", [256, 512], F32, kind="ExternalInput").ap()
    fc1b = nc.dram_tensor("fc1b", [128, 4], F32, kind="ExternalInput").ap()
    fc2w = nc.dram_tensor("fc2w", [512, 256], F32, kind="ExternalInput").ap()
    fc2b = nc.dram_tensor("fc2b", [128, 2], F32, kind="ExternalInput").ap()
    fc3w = nc.dram_tensor("fc3w", [256, 16], F32, kind="ExternalInput").ap()
    fc3b = nc.dram_tensor("fc3b", [16, 1], F32, kind="ExternalInput").ap()
    out = nc.dram_tensor("out", [16, 1], F32, kind="ExternalOutput").ap()
    dbg = {}
    if debug:
        for name, shape in [("h2d", [65, N]), ("h3d", [128, N]), ("h4d", [128, 2, N]),
                            ("jt0", [128, 24]), ("ix0", [128, 160])]:
            dt = U32 if name == "jt0" else (I16 if name == "ix0" else F32)
            dbg[name] = nc.dram_tensor(name, shape, dt, kind="ExternalOutput").ap()

    v_drams = {}

    def run_layer(tc, layer, C, D, hTn, nsq_row, consts, halves, prep=None,
                  aug=False):
        """One EdgeConv layer, software-pipelined over 16 row tiles.

        halves: list of (uT_ap [dh, N], outT_ap [dh, N]) per 128-feature
        half.  prep(t, ppp, psb): emits the NEXT layer's per-tile input prep
        right after h^T columns ts(t) are complete.
        """
        ones1, iota16, MB, pk_bufs, idnh = consts
        EC = len(halves)
        DEPTH = _DEPTHS[layer]   # reduce lag
        Dpad = 128 * EC
        vslice = v_drams[layer]

        with tc.tile_pool(name=f"L{layer}d", bufs=1, space="PSUM") as dps, \
             tc.tile_pool(name=f"L{layer}r", bufs=1, space="PSUM") as rps, \
             tc.tile_pool(name=f"L{layer}pp", bufs=1, space="PSUM") as ppp, \
             tc.tile_pool(name=f"L{layer}sel", bufs=3) as selp, \
             tc.tile_pool(name=f"L{layer}ps", bufs=2) as psb, \
             tc.tile_pool(name=f"L{layer}g", bufs=DEPTH + 1) as gp:
            gbs = {}
            pks = pk_bufs

            def reduce_tile(t):
                # fp16 max tree over the 20 slot blocks (free axis), 2x mode
                gb = gbs.pop(t)
                nc.vector.tensor_tensor(out=gb[:, :, 0:1280], in0=gb[:, :, 0:1280],
                                        in1=gb[:, :, 1280:2560], op=ALU.max)
                nc.vector.tensor_tensor(out=gb[:, :, 0:640], in0=gb[:, :, 0:640],
                                        in1=gb[:, :, 640:1280], op=ALU.max)
                nc.vector.tensor_tensor(out=gb[:, :, 0:256], in0=gb[:, :, 0:256],
                                        in1=gb[:, :, 256:512], op=ALU.max)
                nc.vector.tensor_tensor(out=gb[:, :, 0:128], in0=gb[:, :, 0:128],
                                        in1=gb[:, :, 128:256], op=ALU.max)
                nc.vector.tensor_tensor(out=gb[:, :, 0:128], in0=gb[:, :, 0:128],
                                        in1=gb[:, :, 512:640], op=ALU.max)
                for h, (ut, outT) in enumerate(halves):
                    dh = ut.shape[0]
                    ph = rps.tile([128, 128], F32, name="ph", tag="ph")
                    nc.tensor.matmul(ph[0:dh, :], idnh[0:dh, 0:dh],
                                     gb[0:dh, h, 0:128], start=True, stop=False)
                    nc.tensor.matmul(ph[0:dh, :], idnh[0:dh, 0:dh],
                                     ut[:, ts(t, 128)], start=False, stop=True)
                    nc.scalar.copy(outT[:, ts(t, 128)], ph[0:dh, :])
                if prep is not None:
                    prep(t, (ppp, rps), psb)

            for t in range(NT):
                dp = dps.tile([128, N], F32, name="dp")
                for j in range(4):
                    if aug:
                        # ones+nsq rows ride along in the contraction:
                        # d'ij = h_i.h_j + nsq[j] + nsq[i] (row shift is
                        # ranking-neutral), saving the 1-row accum matmul
                        # (the cost model charges by output rows)
                        nc.tensor.matmul(dp[:, ts(j, 512)],
                                         hTn[0:C + 2, ts(t, 128)],
                                         hTn[0:C + 2, ts(j, 512)],
                                         start=True, stop=True)
                    else:
                        nc.tensor.matmul(dp[:, ts(j, 512)],
                                         hTn[0:C, ts(t, 128)],
                                         hTn[0:C, ts(j, 512)],
                                         start=True, stop=False)
                        nc.tensor.matmul(dp[:, ts(j, 512)], ones1,
                                         nsq_row[:, ts(j, 512)],
                                         start=False, stop=True)

                # pack distances with the column index in the low mantissa
                # bits (bitvec ops exist only on DVE; GPSIMD can't run them
                # and can't read PSUM anyway)
                # selection key: u32 = (fp16(d) << 16) | column index.
                # Act converts the f32 PSUM distances to fp16 in the high
                # u16 halves; the low halves keep the static iota.  The u32
                # compares as f32 exactly like the real distance (ties break
                # by index), so the DVE bit-pack pass disappears.
                packed = pks[t % 2]
                nc.scalar.copy(packed.bitcast(F16)[:, :, 1], dp[:])

                cand = selp.tile([128, 64], F32, name="cand")
                jtp = selp.tile([128, 24], F32, name="jtp")
                jtab = selp.tile([128, 24], U32, name="jtab")
                jtabf = selp.tile([128, 24], F32R, name="jtabf")
                pf = packed.bitcast(F32)
                for c in range(NCH):
                    nc.vector.max(out=cand[:, ts(c, 8)], in_=pf[:, ts(c, CHW), 0])
                nc.vector.max(out=jtp[:, 0:8], in_=cand[:])
                nc.vector.match_replace(out=cand[:], in_to_replace=jtp[:, 0:8],
                                        in_values=cand[:], imm_value=-3.0e38)
                nc.vector.max(out=jtp[:, 8:16], in_=cand[:])
                nc.vector.match_replace(out=cand[:], in_to_replace=jtp[:, 8:16],
                                        in_values=cand[:], imm_value=-3.0e38)
                nc.vector.max(out=jtp[:, 16:24], in_=cand[:])
                ts_imm(nc.vector, out=jtab[:], in0=jtp.bitcast(U32)[:], imm=0xFFFF,
                       op0=ALU.bitwise_and)
                if dbg and layer == 1 and t == 0:
                    nc.sync.dma_start(dbg["jt0"], jtab[:])

                # ---- int16 wrapped index table via PE fold matmuls ----
                nc.scalar.copy(jtabf[:], jtab[:])
                R = rps.tile([128, 8, K], F32, name="R")
                for b in range(8):
                    nc.tensor.matmul(R[:, b, :], MB[:, ts(b, 128)],
                                     jtabf[:, 1:K + 1], start=True, stop=True)
                idx16 = selp.tile([128, K, 8], I16, name="idx16")
                nc.scalar.copy(idx16[:], R[:].transpose([0, 2, 1]))
                if dbg and layer == 1 and t == 0:
                    nc.sync.dma_start(dbg["ix0"], idx16[:])

                gb = gp.tile([128, EC, NI], F16, name="gb")
                gbs[t] = gb
                # single_packet=False: the single-packet ucode path dies on
                # hw above ~1k indices per call
                nc.gpsimd.dma_gather(
                    out_ap=gb[:], in_ap=vslice, idxs_ap=idx16[:],
                    num_idxs=NI, num_idxs_reg=NI, elem_size=Dpad,
                    transpose=True, single_packet=False)
                if layer == 1 and t == 0:
                    # buffer 1's iota init rides behind tile 0's selection so
                    # it doesn't gate the first max8 in the in-order queue
                    ts_imm(nc.vector, out=pk_bufs[1][:, :, 0], in0=iota16[:],
                           imm=0, op0=ALU.bypass)
                if t >= DEPTH:
                    reduce_tile(t - DEPTH)
            for tt in range(NT - DEPTH, NT):
                reduce_tile(tt)

    with TileContext(nc) as tc:
        with tc.tile_pool(name="const", bufs=1) as cp, \
             tc.tile_pool(name="feat", bufs=1) as fp, \
             tc.tile_pool(name="vdram", bufs=1, space="DRAM") as vdp:
            v_drams[1] = v1_in
            v_drams[2] = vdp.tile([N, 128], F16, name="v_dram2")
            v_drams[3] = vdp.tile([N, 256], F16, name="v_dram3")
            MB = cp.tile([128, 1024], F32R)
            idnh = cp.tile([128, 128], F16)
            nc.sync.dma_start(idnh[:], idnh_in)
            ones1f = cp.tile([1, 128], F32)
            nc.vector.memset(ones1f[:], 1.0)
            ones1 = cp.tile([1, 128], F32R)
            nc.scalar.copy(ones1[:], ones1f[:])
            onesColf = cp.tile([128, 1], F32)
            nc.vector.memset(onesColf[:], 1.0)
            onesCol = cp.tile([128, 1], F32R)
            nc.scalar.copy(onesCol[:], onesColf[:])
            iota16 = cp.tile([128, N], U16)
            nc.gpsimd.iota(iota16[:], pattern=[[1, N]], base=0,
                           channel_multiplier=0)

            pk_bufs = [fp.tile([128, N, 2], U16, name=f"pk{_b}")
                       for _b in range(2)]
            hTn1 = fp.tile([3, N], F32R)
            hTn2 = fp.tile([64, N], F32R)
            hT3 = fp.tile([128, N], F32R)
            nsqA = fp.tile([1, N], F32R)
            nsqB = fp.tile([1, N], F32R)
            hT4 = fp.tile([128, 2, N], F32)
            uT = fp.tile([128, 2, N], F16)
            xsq = fp.tile([128, N], F32R)

            ts_imm(nc.vector, out=pk_bufs[0][:, :, 0], in0=iota16[:],
                   imm=0, op0=ALU.bypass)
            nc.sync.dma_start(hTn1[0:3, :], xT)
            nc.sync.dma_start(nsqA[0:1, :], nsq1_in)
            nc.sync.dma_start(MB[:], MB_in)

            fw1 = [fp.tile([128, 512], F32, name=f"fw1_{kk}") for kk in range(2)]
            fw2 = [fp.tile([128, 256], F32, name=f"fw2_{kk}") for kk in range(4)]
            fw3 = [fp.tile([128, 16], F32, name=f"fw3_{kk}") for kk in range(2)]
            fb1 = fp.tile([128, 4], F32)
            fb2 = fp.tile([128, 2], F32)
            fb3 = fp.tile([16, 1], F32)

            with tc.tile_pool(name="wts", bufs=1) as wp:
                w = {}
                for nm, ap_, shape, dt_ in [
                        ("AB2", AB2, [64, 128], F32R), ("BB2", BB2, [64, 128], F32R),
                        ("b2c", b2c, [128, 1], F32),
                        ("AB3", AB3, [128, 256], F32R), ("BB3", BB3, [128, 256], F32R),
                        ("b3c", b3c, [128, 2], F32)]:
                    tl = wp.tile(shape, dt_, name=f"w_{nm}")
                    nc.sync.dma_start(tl[:], ap_)
                    w[nm] = tl

                consts = (ones1[:], iota16[:], MB[:], pk_bufs, idnh[:])

                def make_prep(nl, Cn, Dn, hTnext, BBn, ab_halves, nsq_dst):
                    """Per-tile prep of layer `nl` inputs from hTnext columns.
                    ab_halves: list of (AB_ap [Cn, dh], bcol [dh, 1],
                    uT_dst [dh, N])."""
                    def prep(t, ppp, psb):
                        pv = ppp[1].tile([128, Dn], F32, name=f"pv{nl}",
                                         tag=f"pv{nl}")
                        nc.tensor.matmul(pv[:], hTnext[0:Cn, ts(t, 128)], BBn,
                                         start=True, stop=True)
                        vsb = psb.tile([128, Dn], F16, name=f"vsb{nl}")
                        nc.scalar.copy(vsb[:], pv[:])
                        nc.sync.dma_start(v_drams[nl][ts(t, 128), :], vsb[:])
                        nc.scalar.square(xsq[0:Cn, ts(t, 128)],
                                         hTnext[0:Cn, ts(t, 128)])
                        if t % 4 == 3:
                            j = t // 4
                            for hh, (ab, bcol, ut) in enumerate(ab_halves):
                                dh = ut.shape[0]
                                pu = ppp[0].tile([dh, 512], F32, name=f"pu{nl}",
                                              tag=f"pu{nl}")
                                if hh == 0:
                                    # nsq reduction borrows row 0 of pu
                                    # before pu's own matmul resets it
                                    sqv = pu[0:1, :]
                                    nc.tensor.matmul(sqv, onesCol[0:Cn, 0:1],
                                                     xsq[0:Cn, ts(j, 512)],
                                                     start=True, stop=True)
                                    nc.scalar.activation(
                                        nsq_dst[0:1, ts(j, 512)], sqv,
                                        AF.Copy, scale=-0.5)
                                nc.tensor.matmul(pu[:], ab, hTnext[0:Cn, ts(j, 512)],
                                                 start=True, stop=True)
                                nc.scalar.activation(ut[:, ts(j, 512)], pu[:],
                                                     AF.Identity, bias=bcol,
                                                     scale=1.0)
                    return prep

                nc.sync.dma_start(uT[0:64, 0, :], uT1_in)

                prep2 = make_prep(2, 64, 128, hTn2,
                                  w["BB2"][:],
                                  [(w["AB2"][:], w["b2c"][:, 0:1], uT[:, 0, :])],
                                  nsqB)
                prep3 = make_prep(3, 128, 256, hT3,
                                  w["BB3"][:],
                                  [(w["AB3"][:, 0:128], w["b3c"][:, 0:1],
                                    uT[:, 0, :]),
                                   (w["AB3"][:, 128:256], w["b3c"][:, 1:2],
                                    uT[:, 1, :])],
                                  nsqA)

                run_layer(tc, 1, 3, 64, hTn1, nsqA[0:1, :], consts,
                          [(uT[0:64, 0, :], hTn2[0:64, :])], prep=prep2)
                if dbg:
                    nc.sync.dma_start(dbg["h2d"][0:64, :], hTn2[:].bitcast(F32))

                for kk in range(2):
                    nc.sync.dma_start(fw1[kk][:], fc1w[ts(kk, 128), :])
                    nc.sync.dma_start(fw3[kk][:], fc3w[ts(kk, 128), :])
                for kk in range(4):
                    nc.sync.dma_start(fw2[kk][:], fc2w[ts(kk, 128), :])
                nc.sync.dma_start(fb1[:], fc1b)
                nc.sync.dma_start(fb2[:], fc2b)
                nc.sync.dma_start(fb3[:], fc3b)

                run_layer(tc, 2, 64, 128, hTn2, nsqB[0:1, :], consts,
                          [(uT[:, 0, :], hT3[:])], prep=prep3)
                if dbg:
                    nc.sync.dma_start(dbg["h3d"], hT3[:].bitcast(F32))

                g01 = fp.tile([128, 2], F32)
                g01p = fp.tile([128, 2], F32)

                def prep_g(t, ppp, psb):
                    # bulk of the global max pool runs during tile 15's
                    # gathers; only the last 128 columns remain for the tail
                    if t == 14:
                        nc.vector.tensor_reduce(out=g01[:], in_=hT4[:, :, 0:1920],
                                                axis=AX.X, op=ALU.max)
                    elif t == 15:
                        nc.vector.tensor_reduce(out=g01p[:],
                                                in_=hT4[:, :, 1920:2048],
                                                axis=AX.X, op=ALU.max)
                        nc.vector.tensor_tensor(out=g01[:], in0=g01[:],
                                                in1=g01p[:], op=ALU.max)

                run_layer(tc, 3, 128, 256, hT3, nsqA[0:1, :], consts,
                          [(uT[:, 0, :], hT4[:, 0, :]),
                           (uT[:, 1, :], hT4[:, 1, :])], prep=prep_g)
                if dbg:
                    nc.sync.dma_start(dbg["h4d"], hT4[:])

            # ---------- global max pool + MLP head ----------
            with tc.tile_pool(name="headps", bufs=4, space="PSUM") as hps:
                a1 = [fp.tile([128, 1], F32, name=f"a1_{m}") for m in range(4)]
                for m in range(4):
                    p = hps.tile([128, 1], F32, name="fcp", tag="fcp")
                    nc.tensor.matmul(p[:], fw1[0][:, ts(m, 128)], g01[:, 0:1],
                                     start=True, stop=False)
                    nc.tensor.matmul(p[:], fw1[1][:, ts(m, 128)], g01[:, 1:2],
                                     start=False, stop=True)
                    nc.scalar.activation(a1[m][:], p[:], AF.Relu,
                                         bias=fb1[:, m:m + 1], scale=1.0)
                a2 = [fp.tile([128, 1], F32, name=f"a2_{m}") for m in range(2)]
                for m in range(2):
                    p = hps.tile([128, 1], F32, name="fcp", tag="fcp")
                    for kk in range(4):
                        nc.tensor.matmul(p[:], fw2[kk][:, ts(m, 128)], a1[kk][:],
                                         start=(kk == 0), stop=(kk == 3))
                    nc.scalar.activation(a2[m][:], p[:], AF.Relu,
                                         bias=fb2[:, m:m + 1], scale=1.0)
                p3 = hps.tile([128, 1], F32, name="fcp", tag="fcp")[0:16, :]
                for kk in range(2):
                    nc.tensor.matmul(p3[:], fw3[kk][:], a2[kk][:],
                                     start=(kk == 0), stop=(kk == 1))
                o_sb = fp.tile([16, 1], F32)
                nc.scalar.activation(o_sb[:], p3[:], AF.Identity, bias=fb3[:],
                                     scale=1.0)
                nc.sync.dma_start(out, o_sb[:])

    nc.finalize()
    return nc


def get_nc(debug=False):
    key = bool(debug)
    if key not in _NC_CACHE:
        _NC_CACHE[key] = _builder(debug=debug)
    return _NC_CACHE[key]


def _make_mb():
    # MB[p_in, b*128 + p_out] = 1.0 iff p_in == 16*b + (p_out % 16)
    mb = np.zeros((128, 8, 128), dtype=np.float32)
    for b in range(8):
        for p_out in range(128):
            mb[16 * b + (p_out % 16), b, p_out] = 1.0
    return mb.reshape(128, 1024)


def make_in_maps(x, W1, b1, W2, b2, W3, b3, fc1_w, fc1_b, fc2_w, fc2_b, fc3_w, fc3_b):
    f32 = np.float32
    x = np.asarray(x, f32)
    B = x.shape[0]
    W1, W2, W3 = np.asarray(W1, f32), np.asarray(W2, f32), np.asarray(W3, f32)
    shared = {
        "MB": _make_mb(),
        "idnh": np.eye(128, dtype=np.float16),
        "AB2": np.ascontiguousarray(W2[:64] - W2[64:]),
        "BB2": np.ascontiguousarray(W2[64:]),
        "b2c": np.asarray(b2, f32)[:, None],
        "AB3": np.ascontiguousarray(W3[:128] - W3[128:]),
        "BB3": np.ascontiguousarray(W3[128:]),
        "b3c": np.ascontiguousarray(np.asarray(b3, f32).reshape(2, 128).T),
        "fc1w": np.asarray(fc1_w, f32),
        "fc1b": np.ascontiguousarray(np.asarray(fc1_b, f32).reshape(4, 128).T),
        "fc2w": np.asarray(fc2_w, f32),
        "fc2b": np.ascontiguousarray(np.asarray(fc2_b, f32).reshape(2, 128).T),
        "fc3w": np.pad(np.asarray(fc3_w, f32), ((0, 0), (0, 6))),
        "fc3b": np.pad(np.asarray(fc3_b, f32), (0, 6))[:, None],
    }
    in_maps = []
    for bb in range(B):
        xb = x[bb]
        m = dict(shared)
        m["xT"] = np.ascontiguousarray(xb.T)
        m["nsq1"] = (-0.5 * (xb * xb).sum(-1))[None, :].astype(f32)
        m["v1"] = np.pad((xb @ W1[3:6]).astype(np.float16), ((0, 0), (0, 64)))
        m["uT1"] = np.ascontiguousarray(
            (xb @ (W1[:3] - W1[3:6]) + np.asarray(b1, f32)).T).astype(np.float16)
        in_maps.append(m)
    return in_maps


def kernel(x, k, W1, b1, W2, b2, W3, b3, fc1_w, fc1_b, fc2_w, fc2_b, fc3_w, fc3_b,
           debug=False):
    from concourse import bass_utils
    x = np.asarray(x)
    assert int(k) == 20 and x.shape[1] == 2048 and x.shape[2] == 3
    B = x.shape[0]
    assert B == 8
    nc = get_nc(debug=debug)
    in_maps = make_in_maps(x, W1, b1, W2, b2, W3, b3,
                           fc1_w, fc1_b, fc2_w, fc2_b, fc3_w, fc3_b)
    res = bass_utils.run_bass_kernel_spmd(nc, in_maps, core_ids=list(range(B)))
    outs = np.stack([res.results[bb]["out"][:10, 0] for bb in range(B)], axis=0)
    if debug:
        return outs.astype(np.float32), res
    return outs.astype(np.float32)


# revision 48
# speedup vs baseline: 1.0048x; 1.0048x over previous
"""DGCNN (3x DynamicEdgeConv + global max pool + MLP head) on 8 Trainium2
NeuronCores, data-parallel over the batch (one point cloud per core).

EdgeConv algebra: h_ij = [x_i, x_j - x_i] @ W + b = u_i + v_j with
  u = x @ (Wa - Wb) + b,  v = x @ Wb;  out_i = u_i + max_{j in knn(i)} v_j.
kNN key: d_ij = x_i.x_j - |x_j|^2/2 (nearest = largest, self = row max).

Selection key: u32 = (fp16(d) << 16) | column_index.  The Act engine
converts the f32 PSUM distance row to fp16 directly into the high u16
halves of a persistent buffer whose low halves hold a static iota; the
u32 then compares as f32 exactly like the distance (ties break by index),
so no DVE bit-pack pass is needed.  DVE does top-8 of each 256-wide chunk
(8x max8) -> 64 candidates -> top-24 via 3x max8 + 2x match_replace;
slot 0 = self, slots 1..20 are the 20 nearest.  Chunked selection + fp16
keys are approximate; end-to-end rel err ~1.2e-2 on hw (gate 2e-2).

Gather: ONE dma_gather (SWDGE multi-index ucode, single_packet=False --
the single-packet path dies on hw above ~1k indices) per 128-point row
tile fetches all 20*128 neighbor rows from the fp16 v table in DRAM in
transpose mode, landing feature-major [128d, EC, (slot, point)].  Its
int16 index table wrapped[i%16, i//16] = flat[i], i = slot*128 + p,
replicated across the 8 Q7 core stripes, is produced by 8 one-hot "fold"
matmuls on PE (R[:, b, m] = jtab[16b + p%16, m]) plus an Act-engine
transposing f32->i16 convert.  Neighbor max is a 5-op fp16 tensor_tensor
tree on DVE (2x perf mode); out = u + vmax runs as two fp16 identity
matmuls accumulating into PSUM on the (idle) PE plus an Act copy, keeping
the adds off the DVE critical path.

The reduce of tile t runs DEPTH tiles behind its selection so the fold /
descgen / DMA / sem chain of the gather is fully hidden; the next layer's
v / u^T / nsq are produced inside the tile loop as h^T columns complete.
Global max pool is a free-axis tensor_reduce on [128, 2, N].

Engine budget at ~314us (TimelineSim): DVE ~260 (max8 144, tree ~95),
DMA engines 188 (gathers 175), Act ~175, PE ~150, Pool 93.
"""
import numpy as np

_NC_CACHE = {}
_DEPTHS = {1: 7, 2: 8, 3: 6}

N = 2048
NT = 16          # row tiles of 128 points
NCH = 8          # selection chunks per row (256 wide)
CHW = N // NCH
K = 20
NI = K * 128     # gather indices per row tile



def _builder(debug=False):
    import concourse.bacc as bacc
    import concourse.mybir as mybir
    from concourse.tile import TileContext

    F32 = mybir.dt.float32
    F32R = mybir.dt.float32r
    F16 = mybir.dt.float16
    I16 = mybir.dt.int16
    U32 = mybir.dt.uint32
    U16 = mybir.dt.uint16
    AF = mybir.ActivationFunctionType
    ALU = mybir.AluOpType
    AX = mybir.AxisListType

    def ts(i, s):
        return slice(i * s, (i + 1) * s)

    nc = bacc.Bacc("TRN2", num_devices=8)

    def stt_imm(eng, out, in0, imm, in1, op0, op1):
        """scalar_tensor_tensor with a uint32-typed immediate (the public
        helper hardcodes float32 imm, which the BIR verifier rejects for
        bitvec ops)."""
        return eng.add_instruction(
            mybir.InstTensorScalarPtr(
                name=eng.bass.get_next_instruction_name(),
                is_scalar_tensor_tensor=True,
                op0=op0,
                op1=op1,
                ins=[eng.lower_ap(in0),
                     mybir.ImmediateValue(dtype=mybir.dt.uint32, value=imm),
                     eng.lower_ap(in1)],
                outs=[eng.lower_ap(out)],
            ))

    def ts_imm(eng, out, in0, imm, op0):
        """tensor_scalar with a uint32-typed immediate."""
        return eng.add_instruction(
            mybir.InstTensorScalarPtr(
                name=eng.bass.get_next_instruction_name(),
                op0=op0,
                op1=mybir.AluOpType.bypass,
                ins=[eng.lower_ap(in0),
                     mybir.ImmediateValue(dtype=mybir.dt.uint32, value=imm)],
                outs=[eng.lower_ap(out)],
            ))

    xT = nc.dram_tensor("xT", [3, N], F32R, kind="ExternalInput").ap()
    nsq1_in = nc.dram_tensor("nsq1", [1, N], F32R, kind="ExternalInput").ap()
    v1_in = nc.dram_tensor("v1", [N, 128], F16, kind="ExternalInput").ap()
    uT1_in = nc.dram_tensor("uT1", [64, N], F16, kind="ExternalInput").ap()
    MB_in = nc.dram_tensor("MB", [128, 1024], F32R, kind="ExternalInput").ap()
    idnh_in = nc.dram_tensor("idnh", [128, 128], F16, kind="ExternalInput").ap()
    AB2 = nc.dram_tensor("AB2", [64, 128], F32R, kind="ExternalInput").ap()
    BB2 = nc.dram_tensor("BB2", [64, 128], F32R, kind="ExternalInput").ap()
    b2c = nc.dram_tensor("b2c", [128, 1], F32, kind="ExternalInput").ap()
    AB3 = nc.dram_tensor("AB3", [128, 256], F32R, kind="ExternalInput").ap()
    BB3 = nc.dram_tensor("BB3", [128, 256], F32R, kind="ExternalInput").ap()
    b3c = nc.dram_tensor("b3c", [128, 2], F32, kind="ExternalInput").ap()
    fc1w = nc.dram_tensor("fc1w", [256, 512], F32, kind="ExternalInput").ap()
    fc1b = nc.dram_tensor("fc1b", [128, 4], F32, kind="ExternalInput").ap()
    fc2w = nc.dram_tensor("fc2w", [512, 256], F32, kind="ExternalInput").ap()
    fc2b = nc.dram_tensor("fc2b", [128, 2], F32, kind="ExternalInput").ap()
    fc3w = nc.dram_tensor("fc3w", [256, 16], F32, kind="ExternalInput").ap()
    fc3b = nc.dram_tensor("fc3b", [16, 1], F32, kind="ExternalInput").ap()
    out = nc.dram_tensor("out", [16, 1], F32, kind="ExternalOutput").ap()
    dbg = {}
    if debug:
        for name, shape in [("h2d", [65, N]), ("h3d", [128, N]), ("h4d", [128, 2, N]),
                            ("jt0", [128, 24]), ("ix0", [128, 160])]:
            dt = U32 if name == "jt0" else (I16 if name == "ix0" else F32)
            dbg[name] = nc.dram_tensor(name, shape, dt, kind="ExternalOutput").ap()

    v_drams = {}

    def run_layer(tc, layer, C, D, hTn, nsq_row, consts, halves, prep=None,
                  aug=False):
        """One EdgeConv layer, software-pipelined over 16 row tiles.

        halves: list of (uT_ap [dh, N], outT_ap [dh, N]) per 128-feature
        half.  prep(t, ppp, psb): emits the NEXT layer's per-tile input prep
        right after h^T columns ts(t) are complete.
        """
        ones1, iota16, MB, pk_bufs, idnh = consts
        EC = len(halves)
        DEPTH = _DEPTHS[layer]   # reduce lag
        Dpad = 128 * EC
        vslice = v_drams[layer]

        with tc.tile_pool(name=f"L{layer}d", bufs=1, space="PSUM") as dps, \
             tc.tile_pool(name=f"L{layer}r", bufs=1, space="PSUM") as rps, \
             tc.tile_pool(name=f"L{layer}pp", bufs=1, space="PSUM") as ppp, \
             tc.tile_pool(name=f"L{layer}sel", bufs=3) as selp, \
             tc.tile_pool(name=f"L{layer}ps", bufs=3) as psb, \
             tc.tile_pool(name=f"L{layer}g", bufs=DEPTH + 1) as gp:
            gbs = {}
            pks = pk_bufs

            def reduce_tile(t):
                # fp16 max tree over the 20 slot blocks (free axis), 2x mode
                gb = gbs.pop(t)
                nc.vector.tensor_tensor(out=gb[:, :, 0:1280], in0=gb[:, :, 0:1280],
                                        in1=gb[:, :, 1280:2560], op=ALU.max)
                nc.vector.tensor_tensor(out=gb[:, :, 0:640], in0=gb[:, :, 0:640],
                                        in1=gb[:, :, 640:1280], op=ALU.max)
                nc.vector.tensor_tensor(out=gb[:, :, 0:256], in0=gb[:, :, 0:256],
                                        in1=gb[:, :, 256:512], op=ALU.max)
                nc.vector.tensor_tensor(out=gb[:, :, 0:128], in0=gb[:, :, 0:128],
                                        in1=gb[:, :, 128:256], op=ALU.max)
                nc.vector.tensor_tensor(out=gb[:, :, 0:128], in0=gb[:, :, 0:128],
                                        in1=gb[:, :, 512:640], op=ALU.max)
                for h, (ut, outT) in enumerate(halves):
                    dh = ut.shape[0]
                    ph = rps.tile([128, 128], F32, name="ph", tag="ph")
                    nc.tensor.matmul(ph[0:dh, :], idnh[0:dh, 0:dh],
                                     gb[0:dh, h, 0:128], start=True, stop=False)
                    nc.tensor.matmul(ph[0:dh, :], idnh[0:dh, 0:dh],
                                     ut[:, ts(t, 128)], start=False, stop=True)
                    nc.scalar.copy(outT[:, ts(t, 128)], ph[0:dh, :])
                if prep is not None:
                    prep(t, (ppp, rps), psb)

            for t in range(NT):
                dp = dps.tile([128, N], F32, name="dp")
                for j in range(4):
                    if aug:
                        # ones+nsq rows ride along in the contraction:
                        # d'ij = h_i.h_j + nsq[j] + nsq[i] (row shift is
                        # ranking-neutral), saving the 1-row accum matmul
                        # (the cost model charges by output rows)
                        nc.tensor.matmul(dp[:, ts(j, 512)],
                                         hTn[0:C + 2, ts(t, 128)],
                                         hTn[0:C + 2, ts(j, 512)],
                                         start=True, stop=True)
                    else:
                        nc.tensor.matmul(dp[:, ts(j, 512)],
                                         hTn[0:C, ts(t, 128)],
                                         hTn[0:C, ts(j, 512)],
                                         start=True, stop=False)
                        nc.tensor.matmul(dp[:, ts(j, 512)], ones1,
                                         nsq_row[:, ts(j, 512)],
                                         start=False, stop=True)

                # pack distances with the column index in the low mantissa
                # bits (bitvec ops exist only on DVE; GPSIMD can't run them
                # and can't read PSUM anyway)
                # selection key: u32 = (fp16(d) << 16) | column index.
                # Act converts the f32 PSUM distances to fp16 in the high
                # u16 halves; the low halves keep the static iota.  The u32
                # compares as f32 exactly like the real distance (ties break
                # by index), so the DVE bit-pack pass disappears.
                packed = pks[t % 2]
                nc.scalar.copy(packed.bitcast(F16)[:, :, 1], dp[:])

                cand = selp.tile([128, 64], F32, name="cand")
                jtp = selp.tile([128, 24], F32, name="jtp")
                jtab = selp.tile([128, 24], U32, name="jtab")
                jtabf = selp.tile([128, 24], F32R, name="jtabf")
                pf = packed.bitcast(F32)
                for c in range(NCH):
                    nc.vector.max(out=cand[:, ts(c, 8)], in_=pf[:, ts(c, CHW), 0])
                nc.vector.max(out=jtp[:, 0:8], in_=cand[:])
                nc.vector.match_replace(out=cand[:], in_to_replace=jtp[:, 0:8],
                                        in_values=cand[:], imm_value=-3.0e38)
                nc.vector.max(out=jtp[:, 8:16], in_=cand[:])
                nc.vector.match_replace(out=cand[:], in_to_replace=jtp[:, 8:16],
                                        in_values=cand[:], imm_value=-3.0e38)
                nc.vector.max(out=jtp[:, 16:24], in_=cand[:])
                ts_imm(nc.vector, out=jtab[:], in0=jtp.bitcast(U32)[:], imm=0xFFFF,
                       op0=ALU.bitwise_and)
                if dbg and layer == 1 and t == 0:
                    nc.sync.dma_start(dbg["jt0"], jtab[:])

                # ---- int16 wrapped index table via PE fold matmuls ----
                nc.scalar.copy(jtabf[:], jtab[:])
                R = rps.tile([128, 8, K], F32, name="R")
                for b in range(8):
                    nc.tensor.matmul(R[:, b, :], MB[:, ts(b, 128)],
                                     jtabf[:, 1:K + 1], start=True, stop=True)
                idx16 = selp.tile([128, K, 8], I16, name="idx16")
                nc.scalar.copy(idx16[:], R[:].transpose([0, 2, 1]))
                if dbg and layer == 1 and t == 0:
                    nc.sync.dma_start(dbg["ix0"], idx16[:])

                gb = gp.tile([128, EC, NI], F16, name="gb")
                gbs[t] = gb
                # single_packet=False: the single-packet ucode path dies on
                # hw above ~1k indices per call
                nc.gpsimd.dma_gather(
                    out_ap=gb[:], in_ap=vslice, idxs_ap=idx16[:],
                    num_idxs=NI, num_idxs_reg=NI, elem_size=Dpad,
                    transpose=True, single_packet=False)
                if layer == 1 and t == 0:
                    # buffer 1's iota init rides behind tile 0's selection so
                    # it doesn't gate the first max8 in the in-order queue
                    ts_imm(nc.vector, out=pk_bufs[1][:, :, 0], in0=iota16[:],
                           imm=0, op0=ALU.bypass)
                if t >= DEPTH:
                    reduce_tile(t - DEPTH)
            for tt in range(NT - DEPTH, NT):
                reduce_tile(tt)

    with TileContext(nc) as tc:
        with tc.tile_pool(name="const", bufs=1) as cp, \
             tc.tile_pool(name="feat", bufs=1) as fp, \
             tc.tile_pool(name="vdram", bufs=1, space="DRAM") as vdp:
            v_drams[1] = v1_in
            v_drams[2] = vdp.tile([N, 128], F16, name="v_dram2")
            v_drams[3] = vdp.tile([N, 256], F16, name="v_dram3")
            MB = cp.tile([128, 1024], F32R)
            idnh = cp.tile([128, 128], F16)
            nc.sync.dma_start(idnh[:], idnh_in)
            ones1f = cp.tile([1, 128], F32)
            nc.vector.memset(ones1f[:], 1.0)
            ones1 = cp.tile([1, 128], F32R)
            nc.scalar.copy(ones1[:], ones1f[:])
            onesColf = cp.tile([128, 1], F32)
            nc.vector.memset(onesColf[:], 1.0)
            onesCol = cp.tile([128, 1], F32R)
            nc.scalar.copy(onesCol[:], onesColf[:])
            iota16 = cp.tile([128, N], U16)
            nc.gpsimd.iota(iota16[:], pattern=[[1, N]], base=0,
                           channel_multiplier=0)

            pk_bufs = [fp.tile([128, N, 2], U16, name=f"pk{_b}")
                       for _b in range(2)]
            hTn1 = fp.tile([3, N], F32R)
            hTn2 = fp.tile([64, N], F32R)
            hT3 = fp.tile([128, N], F32R)
            nsqA = fp.tile([1, N], F32R)
            nsqB = fp.tile([1, N], F32R)
            hT4 = fp.tile([128, 2, N], F32)
            uT = fp.tile([128, 2, N], F16)
            xsq = fp.tile([128, N], F32R)

            ts_imm(nc.vector, out=pk_bufs[0][:, :, 0], in0=iota16[:],
                   imm=0, op0=ALU.bypass)
            nc.sync.dma_start(hTn1[0:3, :], xT)
            nc.sync.dma_start(nsqA[0:1, :], nsq1_in)
            nc.sync.dma_start(MB[:], MB_in)

            fw1 = [fp.tile([128, 512], F32, name=f"fw1_{kk}") for kk in range(2)]
            fw2 = [fp.tile([128, 256], F32, name=f"fw2_{kk}") for kk in range(4)]
            fw3 = [fp.tile([128, 16], F32, name=f"fw3_{kk}") for kk in range(2)]
            fb1 = fp.tile([128, 4], F32)
            fb2 = fp.tile([128, 2], F32)
            fb3 = fp.tile([16, 1], F32)

            with tc.tile_pool(name="wts", bufs=1) as wp:
                w = {}
                for nm, ap_, shape, dt_ in [
                        ("AB2", AB2, [64, 128], F32R), ("BB2", BB2, [64, 128], F32R),
                        ("b2c", b2c, [128, 1], F32),
                        ("AB3", AB3, [128, 256], F32R), ("BB3", BB3, [128, 256], F32R),
                        ("b3c", b3c, [128, 2], F32)]:
                    tl = wp.tile(shape, dt_, name=f"w_{nm}")
                    nc.sync.dma_start(tl[:], ap_)
                    w[nm] = tl

                consts = (ones1[:], iota16[:], MB[:], pk_bufs, idnh[:])

                def make_prep(nl, Cn, Dn, hTnext, BBn, ab_halves, nsq_dst):
                    """Per-tile prep of layer `nl` inputs from hTnext columns.
                    ab_halves: list of (AB_ap [Cn, dh], bcol [dh, 1],
                    uT_dst [dh, N])."""
                    def prep(t, ppp, psb):
                        pv = ppp[1].tile([128, Dn], F32, name=f"pv{nl}",
                                         tag=f"pv{nl}")
                        nc.tensor.matmul(pv[:], hTnext[0:Cn, ts(t, 128)], BBn,
                                         start=True, stop=True)
                        vsb = psb.tile([128, Dn], F16, name=f"vsb{nl}")
                        nc.scalar.copy(vsb[:], pv[:])
                        nc.sync.dma_start(v_drams[nl][ts(t, 128), :], vsb[:])
                        nc.scalar.square(xsq[0:Cn, ts(t, 128)],
                                         hTnext[0:Cn, ts(t, 128)])
                        if t % 4 == 3:
                            j = t // 4
                            for hh, (ab, bcol, ut) in enumerate(ab_halves):
                                dh = ut.shape[0]
                                pu = ppp[0].tile([dh, 512], F32, name=f"pu{nl}",
                                              tag=f"pu{nl}")
                                if hh == 0:
                                    # nsq reduction borrows row 0 of pu
                                    # before pu's own matmul resets it
                                    sqv = pu[0:1, :]
                                    nc.tensor.matmul(sqv, onesCol[0:Cn, 0:1],
                                                     xsq[0:Cn, ts(j, 512)],
                                                     start=True, stop=True)
                                    nc.scalar.activation(
                                        nsq_dst[0:1, ts(j, 512)], sqv,
                                        AF.Copy, scale=-0.5)
                                nc.tensor.matmul(pu[:], ab, hTnext[0:Cn, ts(j, 512)],
                                                 start=True, stop=True)
                                nc.scalar.activation(ut[:, ts(j, 512)], pu[:],
                                                     AF.Identity, bias=bcol,
                                                     scale=1.0)
                    return prep

                nc.sync.dma_start(uT[0:64, 0, :], uT1_in)

                prep2 = make_prep(2, 64, 128, hTn2,
                                  w["BB2"][:],
                                  [(w["AB2"][:], w["b2c"][:, 0:1], uT[:, 0, :])],
                                  nsqB)
                prep3 = make_prep(3, 128, 256, hT3,
                                  w["BB3"][:],
                                  [(w["AB3"][:, 0:128], w["b3c"][:, 0:1],
                                    uT[:, 0, :]),
                                   (w["AB3"][:, 128:256], w["b3c"][:, 1:2],
                                    uT[:, 1, :])],
                                  nsqA)

                run_layer(tc, 1, 3, 64, hTn1, nsqA[0:1, :], consts,
                          [(uT[0:64, 0, :], hTn2[0:64, :])], prep=prep2)
                if dbg:
                    nc.sync.dma_start(dbg["h2d"][0:64, :], hTn2[:].bitcast(F32))

                for kk in range(2):
                    nc.sync.dma_start(fw1[kk][:], fc1w[ts(kk, 128), :])
                    nc.sync.dma_start(fw3[kk][:], fc3w[ts(kk, 128), :])
                for kk in range(4):
                    nc.sync.dma_start(fw2[kk][:], fc2w[ts(kk, 128), :])
                nc.sync.dma_start(fb1[:], fc1b)
                nc.sync.dma_start(fb2[:], fc2b)
                nc.sync.dma_start(fb3[:], fc3b)

                run_layer(tc, 2, 64, 128, hTn2, nsqB[0:1, :], consts,
                          [(uT[:, 0, :], hT3[:])], prep=prep3)
                if dbg:
                    nc.sync.dma_start(dbg["h3d"], hT3[:].bitcast(F32))

                g01 = fp.tile([128, 2], F32)
                g01p = fp.tile([128, 2], F32)

                def prep_g(t, ppp, psb):
                    # bulk of the global max pool runs during tile 15's
                    # gathers; only the last 128 columns remain for the tail
                    if t == 14:
                        nc.vector.tensor_reduce(out=g01[:], in_=hT4[:, :, 0:1920],
                                                axis=AX.X, op=ALU.max)
                    elif t == 15:
                        nc.vector.tensor_reduce(out=g01p[:],
                                                in_=hT4[:, :, 1920:2048],
                                                axis=AX.X, op=ALU.max)
                        nc.vector.tensor_tensor(out=g01[:], in0=g01[:],
                                                in1=g01p[:], op=ALU.max)

                run_layer(tc, 3, 128, 256, hT3, nsqA[0:1, :], consts,
                          [(uT[:, 0, :], hT4[:, 0, :]),
                           (uT[:, 1, :], hT4[:, 1, :])], prep=prep_g)
                if dbg:
                    nc.sync.dma_start(dbg["h4d"], hT4[:])

            # ---------- global max pool + MLP head ----------
            with tc.tile_pool(name="headps", bufs=4, space="PSUM") as hps:
                a1 = [fp.tile([128, 1], F32, name=f"a1_{m}") for m in range(4)]
                for m in range(4):
                    p = hps.tile([128, 1], F32, name="fcp", tag="fcp")
                    nc.tensor.matmul(p[:], fw1[0][:, ts(m, 128)], g01[:, 0:1],
                                     start=True, stop=False)
                    nc.tensor.matmul(p[:], fw1[1][:, ts(m, 128)], g01[:, 1:2],
                                     start=False, stop=True)
                    nc.scalar.activation(a1[m][:], p[:], AF.Relu,
                                         bias=fb1[:, m:m + 1], scale=1.0)
                a2 = [fp.tile([128, 1], F32, name=f"a2_{m}") for m in range(2)]
                for m in range(2):
                    p = hps.tile([128, 1], F32, name="fcp", tag="fcp")
                    for kk in range(4):
                        nc.tensor.matmul(p[:], fw2[kk][:, ts(m, 128)], a1[kk][:],
                                         start=(kk == 0), stop=(kk == 3))
                    nc.scalar.activation(a2[m][:], p[:], AF.Relu,
                                         bias=fb2[:, m:m + 1], scale=1.0)
                p3 = hps.tile([128, 1], F32, name="fcp", tag="fcp")[0:16, :]
                for kk in range(2):
                    nc.tensor.matmul(p3[:], fw3[kk][:], a2[kk][:],
                                     start=(kk == 0), stop=(kk == 1))
                o_sb = fp.tile([16, 1], F32)
                nc.scalar.activation(o_sb[:], p3[:], AF.Identity, bias=fb3[:],
                                     scale=1.0)
                nc.sync.dma_start(out, o_sb[:])

    nc.finalize()
    return nc


def get_nc(debug=False):
    key = bool(debug)
    if key not in _NC_CACHE:
        _NC_CACHE[key] = _builder(debug=debug)
    return _NC_CACHE[key]


def _make_mb():
    # MB[p_in, b*128 + p_out] = 1.0 iff p_in == 16*b + (p_out % 16)
    mb = np.zeros((128, 8, 128), dtype=np.float32)
    for b in range(8):
        for p_out in range(128):
            mb[16 * b + (p_out % 16), b, p_out] = 1.0
    return mb.reshape(128, 1024)


def make_in_maps(x, W1, b1, W2, b2, W3, b3, fc1_w, fc1_b, fc2_w, fc2_b, fc3_w, fc3_b):
    f32 = np.float32
    x = np.asarray(x, f32)
    B = x.shape[0]
    W1, W2, W3 = np.asarray(W1, f32), np.asarray(W2, f32), np.asarray(W3, f32)
    shared = {
        "MB": _make_mb(),
        "idnh": np.eye(128, dtype=np.float16),
        "AB2": np.ascontiguousarray(W2[:64] - W2[64:]),
        "BB2": np.ascontiguousarray(W2[64:]),
        "b2c": np.asarray(b2, f32)[:, None],
        "AB3": np.ascontiguousarray(W3[:128] - W3[128:]),
        "BB3": np.ascontiguousarray(W3[128:]),
        "b3c": np.ascontiguousarray(np.asarray(b3, f32).reshape(2, 128).T),
        "fc1w": np.asarray(fc1_w, f32),
        "fc1b": np.ascontiguousarray(np.asarray(fc1_b, f32).reshape(4, 128).T),
        "fc2w": np.asarray(fc2_w, f32),
        "fc2b": np.ascontiguousarray(np.asarray(fc2_b, f32).reshape(2, 128).T),
        "fc3w": np.pad(np.asarray(fc3_w, f32), ((0, 0), (0, 6))),
        "fc3b": np.pad(np.asarray(fc3_b, f32), (0, 6))[:, None],
    }
    in_maps = []
    for bb in range(B):
        xb = x[bb]
        m = dict(shared)
        m["xT"] = np.ascontiguousarray(xb.T)
        m["nsq1"] = (-0.5 * (xb * xb).sum(-1))[None, :].astype(f32)
        m["v1"] = np.pad((xb @ W1[3:6]).astype(np.float16), ((0, 0), (0, 64)))
        m["uT1"] = np.ascontiguousarray(
            (xb @ (W1[:3] - W1[3:6]) + np.asarray(b1, f32)).T).astype(np.float16)
        in_maps.append(m)
    return in_maps


def kernel(x, k, W1, b1, W2, b2, W3, b3, fc1_w, fc1_b, fc2_w, fc2_b, fc3_w, fc3_b,
           debug=False):
    from concourse import bass_utils
    x = np.asarray(x)
    assert int(k) == 20 and x.shape[1] == 2048 and x.shape[2] == 3
    B = x.shape[0]
    assert B == 8
    nc = get_nc(debug=debug)
    in_maps = make_in_maps(x, W1, b1, W2, b2, W3, b3,
                           fc1_w, fc1_b, fc2_w, fc2_b, fc3_w, fc3_b)
    res = bass_utils.run_bass_kernel_spmd(nc, in_maps, core_ids=list(range(B)))
    outs = np.stack([res.results[bb]["out"][:10, 0] for bb in range(B)], axis=0)
    if debug:
        return outs.astype(np.float32), res
    return outs.astype(np.float32)


# revision 49
# speedup vs baseline: 1.0127x; 1.0079x over previous
"""DGCNN (3x DynamicEdgeConv + global max pool + MLP head) on 8 Trainium2
NeuronCores, data-parallel over the batch (one point cloud per core).

EdgeConv algebra: h_ij = [x_i, x_j - x_i] @ W + b = u_i + v_j with
  u = x @ (Wa - Wb) + b,  v = x @ Wb;  out_i = u_i + max_{j in knn(i)} v_j.
kNN key: d_ij = x_i.x_j - |x_j|^2/2 (nearest = largest, self = row max).

Selection key: u32 = (fp16(d) << 16) | column_index.  The Act engine
converts the f32 PSUM distance row to fp16 directly into the high u16
halves of a persistent buffer whose low halves hold a static iota; the
u32 then compares as f32 exactly like the distance (ties break by index),
so no DVE bit-pack pass is needed.  DVE does top-8 of each 256-wide chunk
(8x max8) -> 64 candidates -> top-24 via 3x max8 + 2x match_replace;
slot 0 = self, slots 1..20 are the 20 nearest.  Chunked selection + fp16
keys are approximate; end-to-end rel err ~1.2e-2 on hw (gate 2e-2).

Gather: ONE dma_gather (SWDGE multi-index ucode, single_packet=False --
the single-packet path dies on hw above ~1k indices) per 128-point row
tile fetches all 20*128 neighbor rows from the fp16 v table in DRAM in
transpose mode, landing feature-major [128d, EC, (slot, point)].  Its
int16 index table wrapped[i%16, i//16] = flat[i], i = slot*128 + p,
replicated across the 8 Q7 core stripes, is produced by 8 one-hot "fold"
matmuls on PE (R[:, b, m] = jtab[16b + p%16, m]) plus an Act-engine
transposing f32->i16 convert.  Neighbor max is a 5-op fp16 tensor_tensor
tree on DVE (2x perf mode); out = u + vmax runs as two fp16 identity
matmuls accumulating into PSUM on the (idle) PE plus an Act copy, keeping
the adds off the DVE critical path.

The reduce of tile t runs DEPTH tiles behind its selection so the fold /
descgen / DMA / sem chain of the gather is fully hidden; the next layer's
v / u^T / nsq are produced inside the tile loop as h^T columns complete.
Global max pool is a free-axis tensor_reduce on [128, 2, N].

Engine budget at ~314us (TimelineSim): DVE ~260 (max8 144, tree ~95),
DMA engines 188 (gathers 175), Act ~175, PE ~150, Pool 93.
"""
import numpy as np

_NC_CACHE = {}
_DEPTHS = {1: 7, 2: 7, 3: 6}

N = 2048
NT = 16          # row tiles of 128 points
NCH = 8          # selection chunks per row (256 wide)
CHW = N // NCH
K = 20
NI = K * 128     # gather indices per row tile



def _builder(debug=False):
    import concourse.bacc as bacc
    import concourse.mybir as mybir
    from concourse.tile import TileContext

    F32 = mybir.dt.float32
    F32R = mybir.dt.float32r
    F16 = mybir.dt.float16
    I16 = mybir.dt.int16
    U32 = mybir.dt.uint32
    U16 = mybir.dt.uint16
    AF = mybir.ActivationFunctionType
    ALU = mybir.AluOpType
    AX = mybir.AxisListType

    def ts(i, s):
        return slice(i * s, (i + 1) * s)

    nc = bacc.Bacc("TRN2", num_devices=8)

    def stt_imm(eng, out, in0, imm, in1, op0, op1):
        """scalar_tensor_tensor with a uint32-typed immediate (the public
        helper hardcodes float32 imm, which the BIR verifier rejects for
        bitvec ops)."""
        return eng.add_instruction(
            mybir.InstTensorScalarPtr(
                name=eng.bass.get_next_instruction_name(),
                is_scalar_tensor_tensor=True,
                op0=op0,
                op1=op1,
                ins=[eng.lower_ap(in0),
                     mybir.ImmediateValue(dtype=mybir.dt.uint32, value=imm),
                     eng.lower_ap(in1)],
                outs=[eng.lower_ap(out)],
            ))

    def ts_imm(eng, out, in0, imm, op0):
        """tensor_scalar with a uint32-typed immediate."""
        return eng.add_instruction(
            mybir.InstTensorScalarPtr(
                name=eng.bass.get_next_instruction_name(),
                op0=op0,
                op1=mybir.AluOpType.bypass,
                ins=[eng.lower_ap(in0),
                     mybir.ImmediateValue(dtype=mybir.dt.uint32, value=imm)],
                outs=[eng.lower_ap(out)],
            ))

    xT = nc.dram_tensor("xT", [3, N], F32R, kind="ExternalInput").ap()
    nsq1_in = nc.dram_tensor("nsq1", [1, N], F32R, kind="ExternalInput").ap()
    v1_in = nc.dram_tensor("v1", [N, 128], F16, kind="ExternalInput").ap()
    uT1_in = nc.dram_tensor("uT1", [64, N], F16, kind="ExternalInput").ap()
    MB_in = nc.dram_tensor("MB", [128, 1024], F32R, kind="ExternalInput").ap()
    idnh_in = nc.dram_tensor("idnh", [128, 128], F16, kind="ExternalInput").ap()
    AB2 = nc.dram_tensor("AB2", [64, 128], F32R, kind="ExternalInput").ap()
    BB2 = nc.dram_tensor("BB2", [64, 128], F32R, kind="ExternalInput").ap()
    b2c = nc.dram_tensor("b2c", [128, 1], F32, kind="ExternalInput").ap()
    AB3 = nc.dram_tensor("AB3", [128, 256], F32R, kind="ExternalInput").ap()
    BB3 = nc.dram_tensor("BB3", [128, 256], F32R, kind="ExternalInput").ap()
    b3c = nc.dram_tensor("b3c", [128, 2], F32, kind="ExternalInput").ap()
    fc1w = nc.dram_tensor("fc1w", [256, 512], F32, kind="ExternalInput").ap()
    fc1b = nc.dram_tensor("fc1b", [128, 4], F32, kind="ExternalInput").ap()
    fc2w = nc.dram_tensor("fc2w", [512, 256], F32, kind="ExternalInput").ap()
    fc2b = nc.dram_tensor("fc2b", [128, 2], F32, kind="ExternalInput").ap()
    fc3w = nc.dram_tensor("fc3w", [256, 16], F32, kind="ExternalInput").ap()
    fc3b = nc.dram_tensor("fc3b", [16, 1], F32, kind="ExternalInput").ap()
    out = nc.dram_tensor("out", [16, 1], F32, kind="ExternalOutput").ap()
    dbg = {}
    if debug:
        for name, shape in [("h2d", [65, N]), ("h3d", [128, N]), ("h4d", [128, 2, N]),
                            ("jt0", [128, 24]), ("ix0", [128, 160])]:
            dt = U32 if name == "jt0" else (I16 if name == "ix0" else F32)
            dbg[name] = nc.dram_tensor(name, shape, dt, kind="ExternalOutput").ap()

    v_drams = {}

    def run_layer(tc, layer, C, D, hTn, nsq_row, consts, halves, prep=None,
                  aug=False):
        """One EdgeConv layer, software-pipelined over 16 row tiles.

        halves: list of (uT_ap [dh, N], outT_ap [dh, N]) per 128-feature
        half.  prep(t, ppp, psb): emits the NEXT layer's per-tile input prep
        right after h^T columns ts(t) are complete.
        """
        ones1, iota16, MB, pk_bufs, idnh = consts
        EC = len(halves)
        DEPTH = _DEPTHS[layer]   # reduce lag
        Dpad = 128 * EC
        vslice = v_drams[layer]

        with tc.tile_pool(name=f"L{layer}d", bufs=1, space="PSUM") as dps, \
             tc.tile_pool(name=f"L{layer}r", bufs=1, space="PSUM") as rps, \
             tc.tile_pool(name=f"L{layer}pp", bufs=1, space="PSUM") as ppp, \
             tc.tile_pool(name=f"L{layer}sel", bufs=3) as selp, \
             tc.tile_pool(name=f"L{layer}ps", bufs=4) as psb, \
             tc.tile_pool(name=f"L{layer}g", bufs=DEPTH + 1) as gp:
            gbs = {}
            pks = pk_bufs

            def reduce_tile(t):
                # fp16 max tree over the 20 slot blocks (free axis), 2x mode
                gb = gbs.pop(t)
                nc.vector.tensor_tensor(out=gb[:, :, 0:1280], in0=gb[:, :, 0:1280],
                                        in1=gb[:, :, 1280:2560], op=ALU.max)
                nc.vector.tensor_tensor(out=gb[:, :, 0:640], in0=gb[:, :, 0:640],
                                        in1=gb[:, :, 640:1280], op=ALU.max)
                nc.vector.tensor_tensor(out=gb[:, :, 0:256], in0=gb[:, :, 0:256],
                                        in1=gb[:, :, 256:512], op=ALU.max)
                nc.vector.tensor_tensor(out=gb[:, :, 0:128], in0=gb[:, :, 0:128],
                                        in1=gb[:, :, 128:256], op=ALU.max)
                nc.vector.tensor_tensor(out=gb[:, :, 0:128], in0=gb[:, :, 0:128],
                                        in1=gb[:, :, 512:640], op=ALU.max)
                for h, (ut, outT) in enumerate(halves):
                    dh = ut.shape[0]
                    ph = rps.tile([128, 128], F32, name="ph", tag="ph")
                    nc.tensor.matmul(ph[0:dh, :], idnh[0:dh, 0:dh],
                                     gb[0:dh, h, 0:128], start=True, stop=False)
                    nc.tensor.matmul(ph[0:dh, :], idnh[0:dh, 0:dh],
                                     ut[:, ts(t, 128)], start=False, stop=True)
                    nc.scalar.copy(outT[:, ts(t, 128)], ph[0:dh, :])
                if prep is not None:
                    prep(t, (ppp, rps), psb)

            for t in range(NT):
                dp = dps.tile([128, N], F32, name="dp")
                for j in range(4):
                    if aug:
                        # ones+nsq rows ride along in the contraction:
                        # d'ij = h_i.h_j + nsq[j] + nsq[i] (row shift is
                        # ranking-neutral), saving the 1-row accum matmul
                        # (the cost model charges by output rows)
                        nc.tensor.matmul(dp[:, ts(j, 512)],
                                         hTn[0:C + 2, ts(t, 128)],
                                         hTn[0:C + 2, ts(j, 512)],
                                         start=True, stop=True)
                    else:
                        nc.tensor.matmul(dp[:, ts(j, 512)],
                                         hTn[0:C, ts(t, 128)],
                                         hTn[0:C, ts(j, 512)],
                                         start=True, stop=False)
                        nc.tensor.matmul(dp[:, ts(j, 512)], ones1,
                                         nsq_row[:, ts(j, 512)],
                                         start=False, stop=True)

                # pack distances with the column index in the low mantissa
                # bits (bitvec ops exist only on DVE; GPSIMD can't run them
                # and can't read PSUM anyway)
                # selection key: u32 = (fp16(d) << 16) | column index.
                # Act converts the f32 PSUM distances to fp16 in the high
                # u16 halves; the low halves keep the static iota.  The u32
                # compares as f32 exactly like the real distance (ties break
                # by index), so the DVE bit-pack pass disappears.
                packed = pks[t % 2]
                nc.scalar.copy(packed.bitcast(F16)[:, :, 1], dp[:])

                cand = selp.tile([128, 64], F32, name="cand")
                jtp = selp.tile([128, 24], F32, name="jtp")
                jtab = selp.tile([128, 24], U32, name="jtab")
                jtabf = selp.tile([128, 24], F32R, name="jtabf")
                pf = packed.bitcast(F32)
                for c in range(NCH):
                    nc.vector.max(out=cand[:, ts(c, 8)], in_=pf[:, ts(c, CHW), 0])
                nc.vector.max(out=jtp[:, 0:8], in_=cand[:])
                nc.vector.match_replace(out=cand[:], in_to_replace=jtp[:, 0:8],
                                        in_values=cand[:], imm_value=-3.0e38)
                nc.vector.max(out=jtp[:, 8:16], in_=cand[:])
                nc.vector.match_replace(out=cand[:], in_to_replace=jtp[:, 8:16],
                                        in_values=cand[:], imm_value=-3.0e38)
                nc.vector.max(out=jtp[:, 16:24], in_=cand[:])
                ts_imm(nc.vector, out=jtab[:], in0=jtp.bitcast(U32)[:], imm=0xFFFF,
                       op0=ALU.bitwise_and)
                if dbg and layer == 1 and t == 0:
                    nc.sync.dma_start(dbg["jt0"], jtab[:])

                # ---- int16 wrapped index table via PE fold matmuls ----
                nc.scalar.copy(jtabf[:], jtab[:])
                R = rps.tile([128, 8, K], F32, name="R")
                for b in range(8):
                    nc.tensor.matmul(R[:, b, :], MB[:, ts(b, 128)],
                                     jtabf[:, 1:K + 1], start=True, stop=True)
                idx16 = selp.tile([128, K, 8], I16, name="idx16")
                nc.scalar.copy(idx16[:], R[:].transpose([0, 2, 1]))
                if dbg and layer == 1 and t == 0:
                    nc.sync.dma_start(dbg["ix0"], idx16[:])

                gb = gp.tile([128, EC, NI], F16, name="gb")
                gbs[t] = gb
                # single_packet=False: the single-packet ucode path dies on
                # hw above ~1k indices per call
                nc.gpsimd.dma_gather(
                    out_ap=gb[:], in_ap=vslice, idxs_ap=idx16[:],
                    num_idxs=NI, num_idxs_reg=NI, elem_size=Dpad,
                    transpose=True, single_packet=False)
                if layer == 1 and t == 0:
                    # buffer 1's iota init rides behind tile 0's selection so
                    # it doesn't gate the first max8 in the in-order queue
                    ts_imm(nc.vector, out=pk_bufs[1][:, :, 0], in0=iota16[:],
                           imm=0, op0=ALU.bypass)
                if t >= DEPTH:
                    reduce_tile(t - DEPTH)
            for tt in range(NT - DEPTH, NT):
                reduce_tile(tt)

    with TileContext(nc) as tc:
        with tc.tile_pool(name="const", bufs=1) as cp, \
             tc.tile_pool(name="feat", bufs=1) as fp, \
             tc.tile_pool(name="vdram", bufs=1, space="DRAM") as vdp:
            v_drams[1] = v1_in
            v_drams[2] = vdp.tile([N, 128], F16, name="v_dram2")
            v_drams[3] = vdp.tile([N, 256], F16, name="v_dram3")
            MB = cp.tile([128, 1024], F32R)
            idnh = cp.tile([128, 128], F16)
            nc.sync.dma_start(idnh[:], idnh_in)
            ones1f = cp.tile([1, 128], F32)
            nc.vector.memset(ones1f[:], 1.0)
            ones1 = cp.tile([1, 128], F32R)
            nc.scalar.copy(ones1[:], ones1f[:])
            onesColf = cp.tile([128, 1], F32)
            nc.vector.memset(onesColf[:], 1.0)
            onesCol = cp.tile([128, 1], F32R)
            nc.scalar.copy(onesCol[:], onesColf[:])
            iota16 = cp.tile([128, N], U16)
            nc.gpsimd.iota(iota16[:], pattern=[[1, N]], base=0,
                           channel_multiplier=0)

            pk_bufs = [fp.tile([128, N, 2], U16, name=f"pk{_b}")
                       for _b in range(2)]
            hTn1 = fp.tile([3, N], F32R)
            hTn2 = fp.tile([64, N], F32R)
            hT3 = fp.tile([128, N], F32R)
            nsqA = fp.tile([1, N], F32R)
            nsqB = fp.tile([1, N], F32R)
            hT4 = fp.tile([128, 2, N], F32)
            uT = fp.tile([128, 2, N], F16)
            xsq = fp.tile([128, N], F32R)

            ts_imm(nc.vector, out=pk_bufs[0][:, :, 0], in0=iota16[:],
                   imm=0, op0=ALU.bypass)
            nc.sync.dma_start(hTn1[0:3, :], xT)
            nc.sync.dma_start(nsqA[0:1, :], nsq1_in)
            nc.sync.dma_start(MB[:], MB_in)

            fw1 = [fp.tile([128, 512], F32, name=f"fw1_{kk}") for kk in range(2)]
            fw2 = [fp.tile([128, 256], F32, name=f"fw2_{kk}") for kk in range(4)]
            fw3 = [fp.tile([128, 16], F32, name=f"fw3_{kk}") for kk in range(2)]
            fb1 = fp.tile([128, 4], F32)
            fb2 = fp.tile([128, 2], F32)
            fb3 = fp.tile([16, 1], F32)

            with tc.tile_pool(name="wts", bufs=1) as wp:
                w = {}
                for nm, ap_, shape, dt_ in [
                        ("AB2", AB2, [64, 128], F32R), ("BB2", BB2, [64, 128], F32R),
                        ("b2c", b2c, [128, 1], F32),
                        ("AB3", AB3, [128, 256], F32R), ("BB3", BB3, [128, 256], F32R),
                        ("b3c", b3c, [128, 2], F32)]:
                    tl = wp.tile(shape, dt_, name=f"w_{nm}")
                    nc.sync.dma_start(tl[:], ap_)
                    w[nm] = tl

                consts = (ones1[:], iota16[:], MB[:], pk_bufs, idnh[:])

                def make_prep(nl, Cn, Dn, hTnext, BBn, ab_halves, nsq_dst):
                    """Per-tile prep of layer `nl` inputs from hTnext columns.
                    ab_halves: list of (AB_ap [Cn, dh], bcol [dh, 1],
                    uT_dst [dh, N])."""
                    def prep(t, ppp, psb):
                        pv = ppp[1].tile([128, Dn], F32, name=f"pv{nl}",
                                         tag=f"pv{nl}")
                        nc.tensor.matmul(pv[:], hTnext[0:Cn, ts(t, 128)], BBn,
                                         start=True, stop=True)
                        vsb = psb.tile([128, Dn], F16, name=f"vsb{nl}")
                        nc.scalar.copy(vsb[:], pv[:])
                        nc.sync.dma_start(v_drams[nl][ts(t, 128), :], vsb[:])
                        nc.scalar.square(xsq[0:Cn, ts(t, 128)],
                                         hTnext[0:Cn, ts(t, 128)])
                        if t % 4 == 3:
                            j = t // 4
                            for hh, (ab, bcol, ut) in enumerate(ab_halves):
                                dh = ut.shape[0]
                                pu = ppp[0].tile([dh, 512], F32, name=f"pu{nl}",
                                              tag=f"pu{nl}")
                                if hh == 0:
                                    # nsq reduction borrows row 0 of pu
                                    # before pu's own matmul resets it
                                    sqv = pu[0:1, :]
                                    nc.tensor.matmul(sqv, onesCol[0:Cn, 0:1],
                                                     xsq[0:Cn, ts(j, 512)],
                                                     start=True, stop=True)
                                    nc.scalar.activation(
                                        nsq_dst[0:1, ts(j, 512)], sqv,
                                        AF.Copy, scale=-0.5)
                                nc.tensor.matmul(pu[:], ab, hTnext[0:Cn, ts(j, 512)],
                                                 start=True, stop=True)
                                nc.scalar.activation(ut[:, ts(j, 512)], pu[:],
                                                     AF.Identity, bias=bcol,
                                                     scale=1.0)
                    return prep

                nc.sync.dma_start(uT[0:64, 0, :], uT1_in)

                prep2 = make_prep(2, 64, 128, hTn2,
                                  w["BB2"][:],
                                  [(w["AB2"][:], w["b2c"][:, 0:1], uT[:, 0, :])],
                                  nsqB)
                prep3 = make_prep(3, 128, 256, hT3,
                                  w["BB3"][:],
                                  [(w["AB3"][:, 0:128], w["b3c"][:, 0:1],
                                    uT[:, 0, :]),
                                   (w["AB3"][:, 128:256], w["b3c"][:, 1:2],
                                    uT[:, 1, :])],
                                  nsqA)

                run_layer(tc, 1, 3, 64, hTn1, nsqA[0:1, :], consts,
                          [(uT[0:64, 0, :], hTn2[0:64, :])], prep=prep2)
                if dbg:
                    nc.sync.dma_start(dbg["h2d"][0:64, :], hTn2[:].bitcast(F32))

                for kk in range(2):
                    nc.sync.dma_start(fw1[kk][:], fc1w[ts(kk, 128), :])
                    nc.sync.dma_start(fw3[kk][:], fc3w[ts(kk, 128), :])
                for kk in range(4):
                    nc.sync.dma_start(fw2[kk][:], fc2w[ts(kk, 128), :])
                nc.sync.dma_start(fb1[:], fc1b)
                nc.sync.dma_start(fb2[:], fc2b)
                nc.sync.dma_start(fb3[:], fc3b)

                run_layer(tc, 2, 64, 128, hTn2, nsqB[0:1, :], consts,
                          [(uT[:, 0, :], hT3[:])], prep=prep3)
                if dbg:
                    nc.sync.dma_start(dbg["h3d"], hT3[:].bitcast(F32))

                g01 = fp.tile([128, 2], F32)
                g01p = fp.tile([128, 2], F32)

                def prep_g(t, ppp, psb):
                    # bulk of the global max pool runs during tile 15's
                    # gathers; only the last 128 columns remain for the tail
                    if t == 14:
                        nc.vector.tensor_reduce(out=g01[:], in_=hT4[:, :, 0:1920],
                                                axis=AX.X, op=ALU.max)
                    elif t == 15:
                        nc.vector.tensor_reduce(out=g01p[:],
                                                in_=hT4[:, :, 1920:2048],
                                                axis=AX.X, op=ALU.max)
                        nc.vector.tensor_tensor(out=g01[:], in0=g01[:],
                                                in1=g01p[:], op=ALU.max)

                run_layer(tc, 3, 128, 256, hT3, nsqA[0:1, :], consts,
                          [(uT[:, 0, :], hT4[:, 0, :]),
                           (uT[:, 1, :], hT4[:, 1, :])], prep=prep_g)
                if dbg:
                    nc.sync.dma_start(dbg["h4d"], hT4[:])

            # ---------- global max pool + MLP head ----------
            with tc.tile_pool(name="headps", bufs=4, space="PSUM") as hps:
                a1 = [fp.tile([128, 1], F32, name=f"a1_{m}") for m in range(4)]
                for m in range(4):
                    p = hps.tile([128, 1], F32, name="fcp", tag="fcp")
                    nc.tensor.matmul(p[:], fw1[0][:, ts(m, 128)], g01[:, 0:1],
                                     start=True, stop=False)
                    nc.tensor.matmul(p[:], fw1[1][:, ts(m, 128)], g01[:, 1:2],
                                     start=False, stop=True)
                    nc.scalar.activation(a1[m][:], p[:], AF.Relu,
                                         bias=fb1[:, m:m + 1], scale=1.0)
                a2 = [fp.tile([128, 1], F32, name=f"a2_{m}") for m in range(2)]
                for m in range(2):
                    p = hps.tile([128, 1], F32, name="fcp", tag="fcp")
                    for kk in range(4):
                        nc.tensor.matmul(p[:], fw2[kk][:, ts(m, 128)], a1[kk][:],
                                         start=(kk == 0), stop=(kk == 3))
                    nc.scalar.activation(a2[m][:], p[:], AF.Relu,
                                         bias=fb2[:, m:m + 1], scale=1.0)
                p3 = hps.tile([128, 1], F32, name="fcp", tag="fcp")[0:16, :]
                for kk in range(2):
                    nc.tensor.matmul(p3[:], fw3[kk][:], a2[kk][:],
                                     start=(kk == 0), stop=(kk == 1))
                o_sb = fp.tile([16, 1], F32)
                nc.scalar.activation(o_sb[:], p3[:], AF.Identity, bias=fb3[:],
                                     scale=1.0)
                nc.sync.dma_start(out, o_sb[:])

    nc.finalize()
    return nc


def get_nc(debug=False):
    key = bool(debug)
    if key not in _NC_CACHE:
        _NC_CACHE[key] = _builder(debug=debug)
    return _NC_CACHE[key]


def _make_mb():
    # MB[p_in, b*128 + p_out] = 1.0 iff p_in == 16*b + (p_out % 16)
    mb = np.zeros((128, 8, 128), dtype=np.float32)
    for b in range(8):
        for p_out in range(128):
            mb[16 * b + (p_out % 16), b, p_out] = 1.0
    return mb.reshape(128, 1024)


def make_in_maps(x, W1, b1, W2, b2, W3, b3, fc1_w, fc1_b, fc2_w, fc2_b, fc3_w, fc3_b):
    f32 = np.float32
    x = np.asarray(x, f32)
    B = x.shape[0]
    W1, W2, W3 = np.asarray(W1, f32), np.asarray(W2, f32), np.asarray(W3, f32)
    shared = {
        "MB": _make_mb(),
        "idnh": np.eye(128, dtype=np.float16),
        "AB2": np.ascontiguousarray(W2[:64] - W2[64:]),
        "BB2": np.ascontiguousarray(W2[64:]),
        "b2c": np.asarray(b2, f32)[:, None],
        "AB3": np.ascontiguousarray(W3[:128] - W3[128:]),
        "BB3": np.ascontiguousarray(W3[128:]),
        "b3c": np.ascontiguousarray(np.asarray(b3, f32).reshape(2, 128).T),
        "fc1w": np.asarray(fc1_w, f32),
        "fc1b": np.ascontiguousarray(np.asarray(fc1_b, f32).reshape(4, 128).T),
        "fc2w": np.asarray(fc2_w, f32),
        "fc2b": np.ascontiguousarray(np.asarray(fc2_b, f32).reshape(2, 128).T),
        "fc3w": np.pad(np.asarray(fc3_w, f32), ((0, 0), (0, 6))),
        "fc3b": np.pad(np.asarray(fc3_b, f32), (0, 6))[:, None],
    }
    in_maps = []
    for bb in range(B):
        xb = x[bb]
        m = dict(shared)
        m["xT"] = np.ascontiguousarray(xb.T)
        m["nsq1"] = (-0.5 * (xb * xb).sum(-1))[None, :].astype(f32)
        m["v1"] = np.pad((xb @ W1[3:6]).astype(np.float16), ((0, 0), (0, 64)))
        m["uT1"] = np.ascontiguousarray(
            (xb @ (W1[:3] - W1[3:6]) + np.asarray(b1, f32)).T).astype(np.float16)
        in_maps.append(m)
    return in_maps


def kernel(x, k, W1, b1, W2, b2, W3, b3, fc1_w, fc1_b, fc2_w, fc2_b, fc3_w, fc3_b,
           debug=False):
    from concourse import bass_utils
    x = np.asarray(x)
    assert int(k) == 20 and x.shape[1] == 2048 and x.shape[2] == 3
    B = x.shape[0]
    assert B == 8
    nc = get_nc(debug=debug)
    in_maps = make_in_maps(x, W1, b1, W2, b2, W3, b3,
                           fc1_w, fc1_b, fc2_w, fc2_b, fc3_w, fc3_b)
    res = bass_utils.run_bass_kernel_spmd(nc, in_maps, core_ids=list(range(B)))
    outs = np.stack([res.results[bb]["out"][:10, 0] for bb in range(B)], axis=0)
    if debug:
        return outs.astype(np.float32), res
    return outs.astype(np.float32)


# revision 50
# speedup vs baseline: 1.0134x; 1.0007x over previous
"""DGCNN (3x DynamicEdgeConv + global max pool + MLP head) on 8 Trainium2
NeuronCores, data-parallel over the batch (one point cloud per core).

EdgeConv algebra: h_ij = [x_i, x_j - x_i] @ W + b = u_i + v_j with
  u = x @ (Wa - Wb) + b,  v = x @ Wb;  out_i = u_i + max_{j in knn(i)} v_j.
kNN key: d_ij = x_i.x_j - |x_j|^2/2 (nearest = largest, self = row max).

Selection key: u32 = (fp16(d) << 16) | column_index.  The Act engine
converts the f32 PSUM distance row to fp16 directly into the high u16
halves of a persistent buffer whose low halves hold a static iota; the
u32 then compares as f32 exactly like the distance (ties break by index),
so no DVE bit-pack pass is needed.  DVE does top-8 of each 256-wide chunk
(8x max8) -> 64 candidates -> top-24 via 3x max8 + 2x match_replace;
slot 0 = self, slots 1..20 are the 20 nearest.  Chunked selection + fp16
keys are approximate; end-to-end rel err ~1.2e-2 on hw (gate 2e-2).

Gather: ONE dma_gather (SWDGE multi-index ucode, single_packet=False --
the single-packet path dies on hw above ~1k indices) per 128-point row
tile fetches all 20*128 neighbor rows from the fp16 v table in DRAM in
transpose mode, landing feature-major [128d, EC, (slot, point)].  Its
int16 index table wrapped[i%16, i//16] = flat[i], i = slot*128 + p,
replicated across the 8 Q7 core stripes, is produced by 8 one-hot "fold"
matmuls on PE (R[:, b, m] = jtab[16b + p%16, m]) plus an Act-engine
transposing f32->i16 convert.  Neighbor max is a 5-op fp16 tensor_tensor
tree on DVE (2x perf mode); out = u + vmax runs as two fp16 identity
matmuls accumulating into PSUM on the (idle) PE plus an Act copy, keeping
the adds off the DVE critical path.

The reduce of tile t runs DEPTH tiles behind its selection so the fold /
descgen / DMA / sem chain of the gather is fully hidden; the next layer's
v / u^T / nsq are produced inside the tile loop as h^T columns complete.
Global max pool is a free-axis tensor_reduce on [128, 2, N].

Engine budget at ~314us (TimelineSim): DVE ~260 (max8 144, tree ~95),
DMA engines 188 (gathers 175), Act ~175, PE ~150, Pool 93.
"""
import numpy as np

_NC_CACHE = {}
_DEPTHS = {1: 7, 2: 7, 3: 6}

N = 2048
NT = 16          # row tiles of 128 points
NCH = 8          # selection chunks per row (256 wide)
CHW = N // NCH
K = 20
NI = K * 128     # gather indices per row tile



def _builder(debug=False):
    import concourse.bacc as bacc
    import concourse.mybir as mybir
    from concourse.tile import TileContext

    F32 = mybir.dt.float32
    F32R = mybir.dt.float32r
    F16 = mybir.dt.float16
    I16 = mybir.dt.int16
    U32 = mybir.dt.uint32
    U16 = mybir.dt.uint16
    AF = mybir.ActivationFunctionType
    ALU = mybir.AluOpType
    AX = mybir.AxisListType

    def ts(i, s):
        return slice(i * s, (i + 1) * s)

    nc = bacc.Bacc("TRN2", num_devices=8)

    def stt_imm(eng, out, in0, imm, in1, op0, op1):
        """scalar_tensor_tensor with a uint32-typed immediate (the public
        helper hardcodes float32 imm, which the BIR verifier rejects for
        bitvec ops)."""
        return eng.add_instruction(
            mybir.InstTensorScalarPtr(
                name=eng.bass.get_next_instruction_name(),
                is_scalar_tensor_tensor=True,
                op0=op0,
                op1=op1,
                ins=[eng.lower_ap(in0),
                     mybir.ImmediateValue(dtype=mybir.dt.uint32, value=imm),
                     eng.lower_ap(in1)],
                outs=[eng.lower_ap(out)],
            ))

    def ts_imm(eng, out, in0, imm, op0):
        """tensor_scalar with a uint32-typed immediate."""
        return eng.add_instruction(
            mybir.InstTensorScalarPtr(
                name=eng.bass.get_next_instruction_name(),
                op0=op0,
                op1=mybir.AluOpType.bypass,
                ins=[eng.lower_ap(in0),
                     mybir.ImmediateValue(dtype=mybir.dt.uint32, value=imm)],
                outs=[eng.lower_ap(out)],
            ))

    xT = nc.dram_tensor("xT", [3, N], F32R, kind="ExternalInput").ap()
    nsq1_in = nc.dram_tensor("nsq1", [1, N], F32R, kind="ExternalInput").ap()
    v1_in = nc.dram_tensor("v1", [N, 128], F16, kind="ExternalInput").ap()
    uT1_in = nc.dram_tensor("uT1", [64, N], F16, kind="ExternalInput").ap()
    MB_in = nc.dram_tensor("MB", [128, 1024], F32R, kind="ExternalInput").ap()
    idnh_in = nc.dram_tensor("idnh", [128, 128], F16, kind="ExternalInput").ap()
    AB2 = nc.dram_tensor("AB2", [64, 128], F32R, kind="ExternalInput").ap()
    BB2 = nc.dram_tensor("BB2", [64, 128], F32R, kind="ExternalInput").ap()
    b2c = nc.dram_tensor("b2c", [128, 1], F32, kind="ExternalInput").ap()
    AB3 = nc.dram_tensor("AB3", [128, 256], F32R, kind="ExternalInput").ap()
    BB3 = nc.dram_tensor("BB3", [128, 256], F32R, kind="ExternalInput").ap()
    b3c = nc.dram_tensor("b3c", [128, 2], F32, kind="ExternalInput").ap()
    fc1w = nc.dram_tensor("fc1w", [256, 512], F32, kind="ExternalInput").ap()
    fc1b = nc.dram_tensor("fc1b", [128, 4], F32, kind="ExternalInput").ap()
    fc2w = nc.dram_tensor("fc2w", [512, 256], F32, kind="ExternalInput").ap()
    fc2b = nc.dram_tensor("fc2b", [128, 2], F32, kind="ExternalInput").ap()
    fc3w = nc.dram_tensor("fc3w", [256, 16], F32, kind="ExternalInput").ap()
    fc3b = nc.dram_tensor("fc3b", [16, 1], F32, kind="ExternalInput").ap()
    out = nc.dram_tensor("out", [16, 1], F32, kind="ExternalOutput").ap()
    dbg = {}
    if debug:
        for name, shape in [("h2d", [65, N]), ("h3d", [128, N]), ("h4d", [128, 2, N]),
                            ("jt0", [128, 24]), ("ix0", [128, 160])]:
            dt = U32 if name == "jt0" else (I16 if name == "ix0" else F32)
            dbg[name] = nc.dram_tensor(name, shape, dt, kind="ExternalOutput").ap()

    v_drams = {}

    def run_layer(tc, layer, C, D, hTn, nsq_row, consts, halves, prep=None,
                  aug=False):
        """One EdgeConv layer, software-pipelined over 16 row tiles.

        halves: list of (uT_ap [dh, N], outT_ap [dh, N]) per 128-feature
        half.  prep(t, ppp, psb): emits the NEXT layer's per-tile input prep
        right after h^T columns ts(t) are complete.
        """
        ones1, iota16, MB, pk_bufs, idnh = consts
        EC = len(halves)
        DEPTH = _DEPTHS[layer]   # reduce lag
        Dpad = 128 * EC
        vslice = v_drams[layer]

        with tc.tile_pool(name=f"L{layer}d", bufs=1, space="PSUM") as dps, \
             tc.tile_pool(name=f"L{layer}r", bufs=1, space="PSUM") as rps, \
             tc.tile_pool(name=f"L{layer}pp", bufs=1, space="PSUM") as ppp, \
             tc.tile_pool(name=f"L{layer}sel", bufs=3) as selp, \
             tc.tile_pool(name=f"L{layer}ps", bufs=4) as psb, \
             tc.tile_pool(name=f"L{layer}g", bufs=DEPTH + 1) as gp:
            gbs = {}
            pks = pk_bufs

            def reduce_tile(t):
                # fp16 max tree over the 20 slot blocks (free axis), 2x mode
                gb = gbs.pop(t)
                nc.vector.tensor_tensor(out=gb[:, :, 0:1280], in0=gb[:, :, 0:1280],
                                        in1=gb[:, :, 1280:2560], op=ALU.max)
                nc.vector.tensor_tensor(out=gb[:, :, 0:640], in0=gb[:, :, 0:640],
                                        in1=gb[:, :, 640:1280], op=ALU.max)
                nc.vector.tensor_tensor(out=gb[:, :, 0:256], in0=gb[:, :, 0:256],
                                        in1=gb[:, :, 256:512], op=ALU.max)
                nc.vector.tensor_tensor(out=gb[:, :, 0:128], in0=gb[:, :, 0:128],
                                        in1=gb[:, :, 128:256], op=ALU.max)
                nc.vector.tensor_tensor(out=gb[:, :, 0:128], in0=gb[:, :, 0:128],
                                        in1=gb[:, :, 512:640], op=ALU.max)
                for h, (ut, outT) in enumerate(halves):
                    dh = ut.shape[0]
                    ph = rps.tile([128, 128], F32, name="ph", tag="ph")
                    nc.tensor.matmul(ph[0:dh, :], idnh[0:dh, 0:dh],
                                     gb[0:dh, h, 0:128], start=True, stop=False)
                    nc.tensor.matmul(ph[0:dh, :], idnh[0:dh, 0:dh],
                                     ut[:, ts(t, 128)], start=False, stop=True)
                    nc.scalar.copy(outT[:, ts(t, 128)], ph[0:dh, :])
                if prep is not None:
                    prep(t, (ppp, rps), psb)

            for t in range(NT):
                dp = dps.tile([128, N], F32, name="dp")
                for j in range(4):
                    if aug:
                        # ones+nsq rows ride along in the contraction:
                        # d'ij = h_i.h_j + nsq[j] + nsq[i] (row shift is
                        # ranking-neutral), saving the 1-row accum matmul
                        # (the cost model charges by output rows)
                        nc.tensor.matmul(dp[:, ts(j, 512)],
                                         hTn[0:C + 2, ts(t, 128)],
                                         hTn[0:C + 2, ts(j, 512)],
                                         start=True, stop=True)
                    else:
                        nc.tensor.matmul(dp[:, ts(j, 512)],
                                         hTn[0:C, ts(t, 128)],
                                         hTn[0:C, ts(j, 512)],
                                         start=True, stop=False)
                        nc.tensor.matmul(dp[:, ts(j, 512)], ones1,
                                         nsq_row[:, ts(j, 512)],
                                         start=False, stop=True)

                # pack distances with the column index in the low mantissa
                # bits (bitvec ops exist only on DVE; GPSIMD can't run them
                # and can't read PSUM anyway)
                # selection key: u32 = (fp16(d) << 16) | column index.
                # Act converts the f32 PSUM distances to fp16 in the high
                # u16 halves; the low halves keep the static iota.  The u32
                # compares as f32 exactly like the real distance (ties break
                # by index), so the DVE bit-pack pass disappears.
                packed = pks[t % 3]
                nc.scalar.copy(packed.bitcast(F16)[:, :, 1], dp[:])

                cand = selp.tile([128, 64], F32, name="cand")
                jtp = selp.tile([128, 24], F32, name="jtp")
                jtab = selp.tile([128, 24], U32, name="jtab")
                jtabf = selp.tile([128, 24], F32R, name="jtabf")
                pf = packed.bitcast(F32)
                for c in range(NCH):
                    nc.vector.max(out=cand[:, ts(c, 8)], in_=pf[:, ts(c, CHW), 0])
                nc.vector.max(out=jtp[:, 0:8], in_=cand[:])
                nc.vector.match_replace(out=cand[:], in_to_replace=jtp[:, 0:8],
                                        in_values=cand[:], imm_value=-3.0e38)
                nc.vector.max(out=jtp[:, 8:16], in_=cand[:])
                nc.vector.match_replace(out=cand[:], in_to_replace=jtp[:, 8:16],
                                        in_values=cand[:], imm_value=-3.0e38)
                nc.vector.max(out=jtp[:, 16:24], in_=cand[:])
                ts_imm(nc.vector, out=jtab[:], in0=jtp.bitcast(U32)[:], imm=0xFFFF,
                       op0=ALU.bitwise_and)
                if dbg and layer == 1 and t == 0:
                    nc.sync.dma_start(dbg["jt0"], jtab[:])

                # ---- int16 wrapped index table via PE fold matmuls ----
                nc.scalar.copy(jtabf[:], jtab[:])
                R = rps.tile([128, 8, K], F32, name="R")
                for b in range(8):
                    nc.tensor.matmul(R[:, b, :], MB[:, ts(b, 128)],
                                     jtabf[:, 1:K + 1], start=True, stop=True)
                idx16 = selp.tile([128, K, 8], I16, name="idx16")
                nc.scalar.copy(idx16[:], R[:].transpose([0, 2, 1]))
                if dbg and layer == 1 and t == 0:
                    nc.sync.dma_start(dbg["ix0"], idx16[:])

                gb = gp.tile([128, EC, NI], F16, name="gb")
                gbs[t] = gb
                # single_packet=False: the single-packet ucode path dies on
                # hw above ~1k indices per call
                nc.gpsimd.dma_gather(
                    out_ap=gb[:], in_ap=vslice, idxs_ap=idx16[:],
                    num_idxs=NI, num_idxs_reg=NI, elem_size=Dpad,
                    transpose=True, single_packet=False)
                if layer == 1 and t in (0, 1):
                    # later buffers' iota inits ride behind early selections
                    # so they don't gate the first max8 in the in-order queue
                    ts_imm(nc.vector, out=pk_bufs[t + 1][:, :, 0],
                           in0=iota16[:], imm=0, op0=ALU.bypass)
                if t >= DEPTH:
                    reduce_tile(t - DEPTH)
            for tt in range(NT - DEPTH, NT):
                reduce_tile(tt)

    with TileContext(nc) as tc:
        with tc.tile_pool(name="const", bufs=1) as cp, \
             tc.tile_pool(name="feat", bufs=1) as fp, \
             tc.tile_pool(name="vdram", bufs=1, space="DRAM") as vdp:
            v_drams[1] = v1_in
            v_drams[2] = vdp.tile([N, 128], F16, name="v_dram2")
            v_drams[3] = vdp.tile([N, 256], F16, name="v_dram3")
            MB = cp.tile([128, 1024], F32R)
            idnh = cp.tile([128, 128], F16)
            nc.sync.dma_start(idnh[:], idnh_in)
            ones1f = cp.tile([1, 128], F32)
            nc.vector.memset(ones1f[:], 1.0)
            ones1 = cp.tile([1, 128], F32R)
            nc.scalar.copy(ones1[:], ones1f[:])
            onesColf = cp.tile([128, 1], F32)
            nc.vector.memset(onesColf[:], 1.0)
            onesCol = cp.tile([128, 1], F32R)
            nc.scalar.copy(onesCol[:], onesColf[:])
            iota16 = cp.tile([128, N], U16)
            nc.gpsimd.iota(iota16[:], pattern=[[1, N]], base=0,
                           channel_multiplier=0)

            pk_bufs = [fp.tile([128, N, 2], U16, name=f"pk{_b}")
                       for _b in range(3)]
            hTn1 = fp.tile([3, N], F32R)
            hTn2 = fp.tile([64, N], F32R)
            hT3 = fp.tile([128, N], F32R)
            nsqA = fp.tile([1, N], F32R)
            nsqB = fp.tile([1, N], F32R)
            hT4 = fp.tile([128, 2, N], F32)
            uT = fp.tile([128, 2, N], F16)
            xsq = fp.tile([128, N], F32R)

            ts_imm(nc.vector, out=pk_bufs[0][:, :, 0], in0=iota16[:],
                   imm=0, op0=ALU.bypass)
            nc.sync.dma_start(hTn1[0:3, :], xT)
            nc.sync.dma_start(nsqA[0:1, :], nsq1_in)
            nc.sync.dma_start(MB[:], MB_in)

            fw1 = [fp.tile([128, 512], F32, name=f"fw1_{kk}") for kk in range(2)]
            fw2 = [fp.tile([128, 256], F32, name=f"fw2_{kk}") for kk in range(4)]
            fw3 = [fp.tile([128, 16], F32, name=f"fw3_{kk}") for kk in range(2)]
            fb1 = fp.tile([128, 4], F32)
            fb2 = fp.tile([128, 2], F32)
            fb3 = fp.tile([16, 1], F32)

            with tc.tile_pool(name="wts", bufs=1) as wp:
                w = {}
                for nm, ap_, shape, dt_ in [
                        ("AB2", AB2, [64, 128], F32R), ("BB2", BB2, [64, 128], F32R),
                        ("b2c", b2c, [128, 1], F32),
                        ("AB3", AB3, [128, 256], F32R), ("BB3", BB3, [128, 256], F32R),
                        ("b3c", b3c, [128, 2], F32)]:
                    tl = wp.tile(shape, dt_, name=f"w_{nm}")
                    nc.sync.dma_start(tl[:], ap_)
                    w[nm] = tl

                consts = (ones1[:], iota16[:], MB[:], pk_bufs, idnh[:])

                def make_prep(nl, Cn, Dn, hTnext, BBn, ab_halves, nsq_dst):
                    """Per-tile prep of layer `nl` inputs from hTnext columns.
                    ab_halves: list of (AB_ap [Cn, dh], bcol [dh, 1],
                    uT_dst [dh, N])."""
                    def prep(t, ppp, psb):
                        pv = ppp[1].tile([128, Dn], F32, name=f"pv{nl}",
                                         tag=f"pv{nl}")
                        nc.tensor.matmul(pv[:], hTnext[0:Cn, ts(t, 128)], BBn,
                                         start=True, stop=True)
                        vsb = psb.tile([128, Dn], F16, name=f"vsb{nl}")
                        nc.scalar.copy(vsb[:], pv[:])
                        nc.sync.dma_start(v_drams[nl][ts(t, 128), :], vsb[:])
                        nc.scalar.square(xsq[0:Cn, ts(t, 128)],
                                         hTnext[0:Cn, ts(t, 128)])
                        if t % 4 == 3:
                            j = t // 4
                            for hh, (ab, bcol, ut) in enumerate(ab_halves):
                                dh = ut.shape[0]
                                pu = ppp[0].tile([dh, 512], F32, name=f"pu{nl}",
                                              tag=f"pu{nl}")
                                if hh == 0:
                                    # nsq reduction borrows row 0 of pu
                                    # before pu's own matmul resets it
                                    sqv = pu[0:1, :]
                                    nc.tensor.matmul(sqv, onesCol[0:Cn, 0:1],
                                                     xsq[0:Cn, ts(j, 512)],
                                                     start=True, stop=True)
                                    nc.scalar.activation(
                                        nsq_dst[0:1, ts(j, 512)], sqv,
                                        AF.Copy, scale=-0.5)
                                nc.tensor.matmul(pu[:], ab, hTnext[0:Cn, ts(j, 512)],
                                                 start=True, stop=True)
                                nc.scalar.activation(ut[:, ts(j, 512)], pu[:],
                                                     AF.Identity, bias=bcol,
                                                     scale=1.0)
                    return prep

                nc.sync.dma_start(uT[0:64, 0, :], uT1_in)

                prep2 = make_prep(2, 64, 128, hTn2,
                                  w["BB2"][:],
                                  [(w["AB2"][:], w["b2c"][:, 0:1], uT[:, 0, :])],
                                  nsqB)
                prep3 = make_prep(3, 128, 256, hT3,
                                  w["BB3"][:],
                                  [(w["AB3"][:, 0:128], w["b3c"][:, 0:1],
                                    uT[:, 0, :]),
                                   (w["AB3"][:, 128:256], w["b3c"][:, 1:2],
                                    uT[:, 1, :])],
                                  nsqA)

                run_layer(tc, 1, 3, 64, hTn1, nsqA[0:1, :], consts,
                          [(uT[0:64, 0, :], hTn2[0:64, :])], prep=prep2)
                if dbg:
                    nc.sync.dma_start(dbg["h2d"][0:64, :], hTn2[:].bitcast(F32))

                for kk in range(2):
                    nc.sync.dma_start(fw1[kk][:], fc1w[ts(kk, 128), :])
                    nc.sync.dma_start(fw3[kk][:], fc3w[ts(kk, 128), :])
                for kk in range(4):
                    nc.sync.dma_start(fw2[kk][:], fc2w[ts(kk, 128), :])
                nc.sync.dma_start(fb1[:], fc1b)
                nc.sync.dma_start(fb2[:], fc2b)
                nc.sync.dma_start(fb3[:], fc3b)

                run_layer(tc, 2, 64, 128, hTn2, nsqB[0:1, :], consts,
                          [(uT[:, 0, :], hT3[:])], prep=prep3)
                if dbg:
                    nc.sync.dma_start(dbg["h3d"], hT3[:].bitcast(F32))

                g01 = fp.tile([128, 2], F32)
                g01p = fp.tile([128, 2], F32)

                def prep_g(t, ppp, psb):
                    # bulk of the global max pool runs during tile 15's
                    # gathers; only the last 128 columns remain for the tail
                    if t == 14:
                        nc.vector.tensor_reduce(out=g01[:], in_=hT4[:, :, 0:1920],
                                                axis=AX.X, op=ALU.max)
                    elif t == 15:
                        nc.vector.tensor_reduce(out=g01p[:],
                                                in_=hT4[:, :, 1920:2048],
                                                axis=AX.X, op=ALU.max)
                        nc.vector.tensor_tensor(out=g01[:], in0=g01[:],
                                                in1=g01p[:], op=ALU.max)

                run_layer(tc, 3, 128, 256, hT3, nsqA[0:1, :], consts,
                          [(uT[:, 0, :], hT4[:, 0, :]),
                           (uT[:, 1, :], hT4[:, 1, :])], prep=prep_g)
                if dbg:
                    nc.sync.dma_start(dbg["h4d"], hT4[:])

            # ---------- global max pool + MLP head ----------
            with tc.tile_pool(name="headps", bufs=4, space="PSUM") as hps:
                a1 = [fp.tile([128, 1], F32, name=f"a1_{m}") for m in range(4)]
                for m in range(4):
                    p = hps.tile([128, 1], F32, name="fcp", tag="fcp")
                    nc.tensor.matmul(p[:], fw1[0][:, ts(m, 128)], g01[:, 0:1],
                                     start=True, stop=False)
                    nc.tensor.matmul(p[:], fw1[1][:, ts(m, 128)], g01[:, 1:2],
                                     start=False, stop=True)
                    nc.scalar.activation(a1[m][:], p[:], AF.Relu,
                                         bias=fb1[:, m:m + 1], scale=1.0)
                a2 = [fp.tile([128, 1], F32, name=f"a2_{m}") for m in range(2)]
                for m in range(2):
                    p = hps.tile([128, 1], F32, name="fcp", tag="fcp")
                    for kk in range(4):
                        nc.tensor.matmul(p[:], fw2[kk][:, ts(m, 128)], a1[kk][:],
                                         start=(kk == 0), stop=(kk == 3))
                    nc.scalar.activation(a2[m][:], p[:], AF.Relu,
                                         bias=fb2[:, m:m + 1], scale=1.0)
                p3 = hps.tile([128, 1], F32, name="fcp", tag="fcp")[0:16, :]
                for kk in range(2):
                    nc.tensor.matmul(p3[:], fw3[kk][:], a2[kk][:],
                                     start=(kk == 0), stop=(kk == 1))
                o_sb = fp.tile([16, 1], F32)
                nc.scalar.activation(o_sb[:], p3[:], AF.Identity, bias=fb3[:],
                                     scale=1.0)
                nc.sync.dma_start(out, o_sb[:])

    nc.finalize()
    return nc


def get_nc(debug=False):
    key = bool(debug)
    if key not in _NC_CACHE:
        _NC_CACHE[key] = _builder(debug=debug)
    return _NC_CACHE[key]


def _make_mb():
    # MB[p_in, b*128 + p_out] = 1.0 iff p_in == 16*b + (p_out % 16)
    mb = np.zeros((128, 8, 128), dtype=np.float32)
    for b in range(8):
        for p_out in range(128):
            mb[16 * b + (p_out % 16), b, p_out] = 1.0
    return mb.reshape(128, 1024)


def make_in_maps(x, W1, b1, W2, b2, W3, b3, fc1_w, fc1_b, fc2_w, fc2_b, fc3_w, fc3_b):
    f32 = np.float32
    x = np.asarray(x, f32)
    B = x.shape[0]
    W1, W2, W3 = np.asarray(W1, f32), np.asarray(W2, f32), np.asarray(W3, f32)
    shared = {
        "MB": _make_mb(),
        "idnh": np.eye(128, dtype=np.float16),
        "AB2": np.ascontiguousarray(W2[:64] - W2[64:]),
        "BB2": np.ascontiguousarray(W2[64:]),
        "b2c": np.asarray(b2, f32)[:, None],
        "AB3": np.ascontiguousarray(W3[:128] - W3[128:]),
        "BB3": np.ascontiguousarray(W3[128:]),
        "b3c": np.ascontiguousarray(np.asarray(b3, f32).reshape(2, 128).T),
        "fc1w": np.asarray(fc1_w, f32),
        "fc1b": np.ascontiguousarray(np.asarray(fc1_b, f32).reshape(4, 128).T),
        "fc2w": np.asarray(fc2_w, f32),
        "fc2b": np.ascontiguousarray(np.asarray(fc2_b, f32).reshape(2, 128).T),
        "fc3w": np.pad(np.asarray(fc3_w, f32), ((0, 0), (0, 6))),
        "fc3b": np.pad(np.asarray(fc3_b, f32), (0, 6))[:, None],
    }
    in_maps = []
    for bb in range(B):
        xb = x[bb]
        m = dict(shared)
        m["xT"] = np.ascontiguousarray(xb.T)
        m["nsq1"] = (-0.5 * (xb * xb).sum(-1))[None, :].astype(f32)
        m["v1"] = np.pad((xb @ W1[3:6]).astype(np.float16), ((0, 0), (0, 64)))
        m["uT1"] = np.ascontiguousarray(
            (xb @ (W1[:3] - W1[3:6]) + np.asarray(b1, f32)).T).astype(np.float16)
        in_maps.append(m)
    return in_maps


def kernel(x, k, W1, b1, W2, b2, W3, b3, fc1_w, fc1_b, fc2_w, fc2_b, fc3_w, fc3_b,
           debug=False):
    from concourse import bass_utils
    x = np.asarray(x)
    assert int(k) == 20 and x.shape[1] == 2048 and x.shape[2] == 3
    B = x.shape[0]
    assert B == 8
    nc = get_nc(debug=debug)
    in_maps = make_in_maps(x, W1, b1, W2, b2, W3, b3,
                           fc1_w, fc1_b, fc2_w, fc2_b, fc3_w, fc3_b)
    res = bass_utils.run_bass_kernel_spmd(nc, in_maps, core_ids=list(range(B)))
    outs = np.stack([res.results[bb]["out"][:10, 0] for bb in range(B)], axis=0)
    if debug:
        return outs.astype(np.float32), res
    return outs.astype(np.float32)
